# revision 11
# baseline (speedup 1.0000x reference)
"""Luong local-p attention (scaled-dot, gaussian window) on 8 trn2 cores.

Strategy (data-parallel over batch, 2 examples/core):
  - Host: transpose source_hidden_states to [H, S] per example so the score
    matmul can contract over H on the TensorEngine with the target vector
    replicated as the stationary operand (scores come out replicated across
    all 128 partitions, which is exactly the layout the windowed context
    multiply needs). Ships a bf16 copy (streamed once for scores/softmax
    stats) and keeps the fp32 copy for the window re-read.
  - p = S*sigmoid(v_p . tanh(W_p^T t + b_p) + b_v) is column-sharded across
    the 8 cores: each core computes 128 of the 1024 tanh columns for ALL 16
    examples against its W_p column slice (fp32 PE matmul), dots them with
    its v_p slice, and a ReduceScatter(add) hands every core the finished
    logits for exactly its 2 examples.
  - Per example: scores[s] = (src[s,:] . t)/sqrt(H) on the PE in bf16,
    psum-accumulated over 8 h-chunks; softmax max + denominator Z over the
    full S; exp(score - m) is saved while computing Z. The gaussian window
    [s0, s0+512), s0 = clamp(floor(p)-256, 0, S-512), covers every position
    whose gaussian factor exceeds ~1e-14, so the context reduces over the
    window only: fp32 window columns are re-fetched with a dynamic-offset
    DMA and multiplied by exp(score-m)*gauss/Z, spread across
    GPSIMD/DVE (multiplies) and ACT/DVE (free-dim reductions).
"""

import numpy as np

N_CORES = 8
B, S, H = 16, 4096, 1024
BEX = B // N_CORES  # examples per core
NH = H // 128  # h-chunks of 128 partitions
NSB = S // 512  # s-blocks of 512
WIN = 512
SCALE = 1.0 / 32.0  # 1/sqrt(H)
GEXP = -1.0 / 2048.0  # -1/(2*sigma^2), sigma = WINDOW/2 = 32
S0MAX = float(S - WIN)

_CACHE = {}


def _build():
    import concourse.bacc as bacc
    import concourse.bass as bass
    import concourse.mybir as mybir
    import concourse.tile as tile

    f32 = mybir.dt.float32
    bf16 = mybir.dt.bfloat16
    i32 = mybir.dt.int32
    AF = mybir.ActivationFunctionType
    OP = mybir.AluOpType
    AX = mybir.AxisListType
    ET = mybir.EngineType
    ds = bass.ds

    nc = bacc.Bacc("TRN2", target_bir_lowering=False, debug=False, num_devices=N_CORES)
    srcT = nc.dram_tensor("srcT", [BEX, H, S], f32, kind="ExternalInput").ap()
    srcTb = nc.dram_tensor("srcTb", [BEX, H, S], bf16, kind="ExternalInput").ap()
    tgt = nc.dram_tensor("tgt", [BEX, H], f32, kind="ExternalInput").ap()
    tgta = nc.dram_tensor("tgta", [B, H], f32, kind="ExternalInput").ap()
    wps = nc.dram_tensor("wps", [H, 128], f32, kind="ExternalInput").ap()
    vps = nc.dram_tensor("vps", [1, 128], f32, kind="ExternalInput").ap()
    bps = nc.dram_tensor("bps", [1, 128], f32, kind="ExternalInput").ap()
    bv = nc.dram_tensor("bv", [1, 1], f32, kind="ExternalInput").ap()
    out = nc.dram_tensor("out", [BEX, NH, 128], f32, kind="ExternalOutput").ap()
    scr_sp = nc.dram_tensor("scr_sp", [BEX, 1], f32).ap()
    cc_in = nc.dram_tensor("cc_in", [B, 1], f32).ap()
    cc_out = nc.dram_tensor("cc_out", [BEX, 1], f32).ap()

    with tile.TileContext(nc) as tc:
        with (
            tc.tile_pool(name="cpool", bufs=1) as cpool,
            tc.tile_pool(name="wpool", bufs=1) as wpool,
            tc.tile_pool(name="spool", bufs=5) as spool,
            tc.tile_pool(name="winpool", bufs=10) as winpool,
            tc.tile_pool(name="mpool", bufs=2) as mpool,
            tc.tile_pool(name="psB", bufs=1, space="PSUM") as psB,
        ):
            # ---------------- setup: tiny DMAs + stationary operands ---------
            tT = []
            tTa = []
            for c in range(NH):
                t_ = cpool.tile([128, BEX], f32, tag=f"tT{c}")
                nc.sync.dma_start(t_[:], tgt[0:BEX, c * 128 : (c + 1) * 128].transpose([1, 0]))
                tT.append(t_)
                ta = cpool.tile([128, B], f32, tag=f"tTa{c}")
                nc.sync.dma_start(ta[:], tgta[0:B, c * 128 : (c + 1) * 128].transpose([1, 0]))
                tTa.append(ta)

            wts = []
            for c in range(NH):
                wt = wpool.tile([128, 128], f32, tag=f"w{c}")
                nc.sync.dma_start(wt[:], wps[c * 128 : (c + 1) * 128, :])
                wts.append(wt)

            bps_b = cpool.tile([B, 128], f32, tag="bps_b")
            nc.sync.dma_start(bps_b[:], bps[0:1, :].to_broadcast((B, 128)))
            vps_b = cpool.tile([B, 128], f32, tag="vps_b")
            nc.sync.dma_start(vps_b[:], vps[0:1, :].to_broadcast((B, 128)))
            bv_sb = cpool.tile([BEX, 1], f32, tag="bv_sb")
            for e in range(BEX):
                nc.sync.dma_start(bv_sb[e : e + 1, :], bv[0:1, :])

            zeros = cpool.tile([128, 128], f32, tag="zeros")
            nc.vector.memset(zeros[:], 0.0)
            t_rep16 = []
            for e in range(BEX):
                r16s = []
                for c in range(NH):
                    r32 = mpool.tile([128, 128], f32, tag="t_rep32", name=f"tr32_{e}_{c}")
                    nc.scalar.activation(
                        r32[:], zeros[:], AF.Identity, bias=tT[c][:, e : e + 1], scale=1.0
                    )
                    r16 = cpool.tile([128, 128], bf16, tag=f"t_rep16_{e}_{c}")
                    nc.vector.tensor_copy(r16[:], r32[:])
                    r16s.append(r16)
                t_rep16.append(r16s)

            iota_i = cpool.tile([128, WIN], i32, tag="iota_i")
            nc.gpsimd.iota(iota_i[:], pattern=[[1, WIN]], base=0, channel_multiplier=0)
            iota_f = cpool.tile([128, WIN], f32, tag="iota_f")
            nc.vector.tensor_copy(iota_f[:], iota_i[:])

            # ---------------- phase 0: sharded p computation -----------------
            ps_hp = psB.tile([B, 128], f32, tag="hp", name="hp_ps")
            for c in range(NH):
                nc.tensor.matmul(
                    ps_hp[:], tTa[c][:], wts[c][:], start=(c == 0), stop=(c == NH - 1)
                )
            hp_cols = cpool.tile([B, 128], f32, tag="hp_cols")
            nc.vector.tensor_tensor(hp_cols[:], ps_hp[:], bps_b[:], OP.add)
            nc.scalar.activation(hp_cols[:], hp_cols[:], AF.Tanh)
            hv = cpool.tile([B, 128], f32, tag="hv")
            nc.vector.tensor_tensor(hv[:], hp_cols[:], vps_b[:], OP.mult)
            partial = cpool.tile([B, 1], f32, tag="partial")
            nc.vector.tensor_reduce(partial[:], hv[:], AX.X, OP.add)
            nc.sync.dma_start(cc_in[:], partial[:])
            nc.gpsimd.collective_compute(
                "ReduceScatter",
                OP.add,
                replica_groups=[list(range(N_CORES))],
                ins=[cc_in[:]],
                outs=[cc_out[:]],
            )
            pre2 = cpool.tile([BEX, 1], f32, tag="pre2")
            nc.sync.dma_start(pre2[:], cc_out[:])

            pv = cpool.tile([BEX, 1], f32, tag="pv")
            nc.scalar.activation(pv[:], pre2[:], AF.Sigmoid, bias=bv_sb[:], scale=1.0)
            nc.vector.tensor_scalar(pv[:], pv[:], float(S), None, OP.mult)

            s0f = cpool.tile([BEX, 1], f32, tag="s0f")
            nc.vector.tensor_scalar(s0f[:], pv[:], 256.0, None, OP.subtract)
            nc.vector.tensor_scalar(s0f[:], s0f[:], 0.0, S0MAX, OP.max, OP.min)
            s0i = cpool.tile([BEX, 1], i32, tag="s0i")
            nc.vector.tensor_copy(s0i[:], s0f[:])
            s0ff = cpool.tile([BEX, 1], f32, tag="s0ff")
            nc.vector.tensor_copy(s0ff[:], s0i[:])

            spd = cpool.tile([BEX, 1], f32, tag="spd")
            nc.vector.tensor_tensor(spd[:], s0ff[:], pv[:], OP.subtract)
            nc.sync.dma_start(scr_sp[:], spd[:])

            s0_regs = []
            for e in range(BEX):
                s0_regs.append(
                    nc.values_load(
                        s0i[e : e + 1, 0:1],
                        engines=[ET.SP, ET.DVE],
                        min_val=0,
                        max_val=int(S0MAX),
                        skip_runtime_bounds_check=True,
                    )
                )

            # gaussian window factors per example: exp(-(s0 + f - p)^2/(2 s^2))
            gauss = []
            for e in range(BEX):
                sp_b = cpool.tile([128, 1], f32, tag=f"sp_b{e}")
                nc.sync.dma_start(sp_b[:], scr_sp[e : e + 1, 0:1].to_broadcast((128, 1)))
                d = mpool.tile([128, WIN], f32, tag="d", name=f"d_{e}")
                nc.vector.tensor_scalar(d[:], iota_f[:], sp_b[:], None, OP.add)
                nc.scalar.activation(d[:], d[:], AF.Square)
                g = cpool.tile([128, WIN], f32, tag=f"gauss{e}")
                nc.scalar.activation(g[:], d[:], AF.Exp, scale=GEXP)
                gauss.append(g)

            # ---------------- per-example phases -----------------------------
            def scores_phase(e):
                ps = []
                for k in range(NSB):
                    tag = "hp" if k == NSB - 1 else f"sc{k}"
                    ps.append(psB.tile([128, 512], f32, tag=tag, name=f"sc{k}_{e}"))
                for c in range(NH):
                    big = spool.tile([128, S], bf16, tag="stream", name=f"big_{e}_{c}")
                    nc.sync.dma_start(big[:], srcTb[e, c * 128 : (c + 1) * 128, :])
                    for k in range(NSB):
                        nc.tensor.matmul(
                            ps[k][:],
                            t_rep16[e][c][:],
                            big[:, k * 512 : (k + 1) * 512],
                            start=(c == 0),
                            stop=(c == NH - 1),
                        )
                return ps

            def stats_phase(e, ps):
                # softmax stats over full S (scores are replicated across rows)
                mx8 = mpool.tile([128, NSB], f32, tag="mx8", name=f"mx8_{e}")
                for k in range(NSB):
                    nc.vector.tensor_reduce(mx8[:, k : k + 1], ps[k][:], AX.X, OP.max)
                m = mpool.tile([128, 1], f32, tag="m", name=f"m_{e}")
                nc.vector.tensor_reduce(m[:], mx8[:], AX.X, OP.max)
                bias_m = mpool.tile([128, 1], f32, tag="bias_m", name=f"bias_m_{e}", bufs=2)
                nc.vector.tensor_scalar(bias_m[:], m[:], -SCALE, None, OP.mult)

                sums8 = mpool.tile([128, NSB], f32, tag="sums8", name=f"sums8_{e}")
                expsc = mpool.tile([128, S], f32, tag="expsc", name=f"expsc_{e}", bufs=2)
                for k in range(NSB):
                    nc.scalar.activation(
                        expsc[:, k * 512 : (k + 1) * 512],
                        ps[k][:],
                        AF.Exp,
                        bias=bias_m[:],
                        scale=SCALE,
                        accum_out=sums8[:, k : k + 1],
                    )
                z = mpool.tile([128, 1], f32, tag="z", name=f"z_{e}")
                nc.vector.tensor_reduce(z[:], sums8[:], AX.X, OP.add)
                rz = mpool.tile([128, 1], f32, tag="rz", name=f"rz_{e}", bufs=2)
                nc.vector.reciprocal(rz[:], z[:])
                return expsc, rz

            def window_phase(e, s0_reg, gauss_e, expsc, rz):
                wins = []
                for c in range(NH):
                    win = winpool.tile([128, WIN], f32, tag="win", name=f"win_{e}_{c}")
                    nc.sync.dma_start(
                        win[:], srcT[e, c * 128 : (c + 1) * 128, ds(s0_reg, WIN)]
                    )
                    wins.append(win)

                attnw = mpool.tile([128, WIN], f32, tag="attnw", name=f"attnw_{e}")
                nc.vector.tensor_tensor(
                    attnw[:], expsc[:, ds(s0_reg, WIN)], gauss_e[:], OP.mult
                )

                ctx = mpool.tile([128, NH], f32, tag="ctx", name=f"ctx_{e}")
                for c in range(NH):
                    scr = mpool.tile(
                        [128, WIN], f32, tag="scr512", name=f"scr_{e}_{c}", bufs=4
                    )
                    if c % 2 == 0:
                        nc.vector.tensor_tensor(scr[:], wins[c][:], attnw[:], OP.mult)
                        ejc = mpool.tile(
                            [128, WIN], f32, tag="ctxjunk", name=f"cj_{e}_{c}", bufs=2
                        )
                        nc.scalar.activation(
                            ejc[:], scr[:], AF.Identity, accum_out=ctx[:, c : c + 1]
                        )
                    else:
                        nc.gpsimd.tensor_tensor(scr[:], wins[c][:], attnw[:], OP.mult)
                        nc.vector.tensor_reduce(ctx[:, c : c + 1], scr[:], AX.X, OP.add)
                nc.vector.tensor_scalar(ctx[:], ctx[:], rz[:], None, OP.mult)
                nc.sync.dma_start(out[e].transpose([1, 0]), ctx[:])

            ps0 = scores_phase(0)
            ex0_stats = stats_phase(0, ps0)
            window_phase(0, s0_regs[0], gauss[0], *ex0_stats)
            ps1 = scores_phase(1)
            ex1_stats = stats_phase(1, ps1)
            window_phase(1, s0_regs[1], gauss[1], *ex1_stats)

    nc.compile()
    return nc


def _get_nc():
    if "nc" not in _CACHE:
        _CACHE["nc"] = _build()
    return _CACHE["nc"]


def _make_in_maps(src, tgt, wp, bp, vp, bv):
    import ml_dtypes

    srcT = np.ascontiguousarray(src.transpose(0, 2, 1))  # [B, H, S]
    srcTb = srcT.astype(ml_dtypes.bfloat16)
    in_maps = []
    for k in range(N_CORES):
        lo, hi = k * BEX, (k + 1) * BEX
        jlo, jhi = k * 128, (k + 1) * 128
        in_maps.append(
            {
                "srcT": srcT[lo:hi],
                "srcTb": srcTb[lo:hi],
                "tgt": np.ascontiguousarray(tgt[lo:hi]),
                "tgta": tgt,
                "wps": np.ascontiguousarray(wp[:, jlo:jhi]),
                "vps": np.ascontiguousarray(vp[:, jlo:jhi]),
                "bps": np.ascontiguousarray(bp[:, jlo:jhi]),
                "bv": bv,
            }
        )
    return in_maps


def kernel(source_hidden_states, target_hidden_state, W_p, b_p, v_p, b_v):
    from concourse.bass_utils import run_bass_kernel_spmd

    src = np.asarray(source_hidden_states, dtype=np.float32)
    tgt = np.asarray(target_hidden_state, dtype=np.float32)
    wp = np.asarray(W_p, dtype=np.float32)
    bp = np.asarray(b_p, dtype=np.float32).reshape(1, H)
    vp = np.asarray(v_p, dtype=np.float32).reshape(1, H)
    bv = np.asarray(b_v, dtype=np.float32).reshape(1, 1)

    nc = _get_nc()
    in_maps = _make_in_maps(src, tgt, wp, bp, vp, bv)
    r = run_bass_kernel_spmd(nc, in_maps, list(range(N_CORES)))
    outs = [r.results[k]["out"].reshape(BEX, H) for k in range(N_CORES)]
    return np.concatenate(outs, axis=0)


# revision 15
# speedup vs baseline: 1.6708x; 1.6708x over previous
"""Luong local-p attention (scaled-dot, gaussian window) on 8 trn2 cores.

Strategy (data-parallel over batch, 2 examples/core):
  - Host: transpose source_hidden_states to [H, S] per example so the score
    matmul can contract over H on the TensorEngine with the target vector
    replicated as the stationary operand (scores come out replicated across
    all 128 partitions, which is exactly the layout the windowed context
    multiply needs). Ships a bf16 copy (streamed once for scores/softmax
    denominator) and keeps the fp32 copy for the window re-read.
  - Device per example:
      p = S*sigmoid(v_p . tanh(W_p^T t + b_p) + b_v)   (fp32 PE matmul + ACT)
      scores[s] = (src[s,:] . t)/sqrt(H)                (bf16 PE, psum-acc)
      softmax denominator Z over full S with a CONSTANT shift of -8 instead
      of the max (scores are ~N(0,1); fp32 range makes a computed max
      unnecessary, and the constant shift cancels exactly in the ratio)
      window [s0, s0+320), s0 = clamp(floor(p)-160, 0, S-320) covers every
      position whose gaussian factor exceeds ~3.7e-6 (5 sigma); window
      scores are recomputed in fp32 from the re-fetched fp32 window columns
      so the attention weights that matter are fp32-accurate. Context =
      windowed multiply-reduce spread across GPSIMD/DVE (multiplies) and
      ACT/DVE (free-dim reductions).
"""

import numpy as np

N_CORES = 8
B, S, H = 16, 4096, 1024
BEX = B // N_CORES  # examples per core
NH = H // 128  # h-chunks of 128 partitions
NSB = S // 512  # s-blocks of 512
WIN = 320
SCALE = 1.0 / 32.0  # 1/sqrt(H)
GEXP = -1.0 / 2048.0  # -1/(2*sigma^2), sigma = WINDOW/2 = 32
EBIAS = -8.0  # constant softmax shift
S0MAX = float(S - WIN)

_CACHE = {}


def _build():
    import concourse.bacc as bacc
    import concourse.bass as bass
    import concourse.mybir as mybir
    import concourse.tile as tile

    f32 = mybir.dt.float32
    bf16 = mybir.dt.bfloat16
    i32 = mybir.dt.int32
    AF = mybir.ActivationFunctionType
    OP = mybir.AluOpType
    AX = mybir.AxisListType
    ET = mybir.EngineType
    ds = bass.ds

    nc = bacc.Bacc("TRN2", target_bir_lowering=False, debug=False, num_devices=N_CORES)
    srcT = nc.dram_tensor("srcT", [BEX, H, S], f32, kind="ExternalInput").ap()
    srcTb = nc.dram_tensor("srcTb", [BEX, H, S], bf16, kind="ExternalInput").ap()
    tgt = nc.dram_tensor("tgt", [BEX, H], f32, kind="ExternalInput").ap()
    wp = nc.dram_tensor("wp", [H, H], f32, kind="ExternalInput").ap()
    vp = nc.dram_tensor("vp", [1, H], f32, kind="ExternalInput").ap()
    bp = nc.dram_tensor("bp", [1, H], f32, kind="ExternalInput").ap()
    bv = nc.dram_tensor("bv", [1, 1], f32, kind="ExternalInput").ap()
    out = nc.dram_tensor("out", [BEX, NH, 128], f32, kind="ExternalOutput").ap()
    scr_sp = nc.dram_tensor("scr_sp", [BEX, 1], f32).ap()

    with tile.TileContext(nc) as tc:
        with (
            tc.tile_pool(name="cpool", bufs=1) as cpool,
            tc.tile_pool(name="wpool", bufs=8) as wpool,
            tc.tile_pool(name="spool", bufs=5) as spool,
            tc.tile_pool(name="winpool", bufs=10) as winpool,
            tc.tile_pool(name="mpool", bufs=2) as mpool,
            tc.tile_pool(name="psB", bufs=1, space="PSUM") as psB,
        ):
            # ---------------- setup: tiny DMAs + stationary operands ---------
            tT = []
            for c in range(NH):
                t_ = cpool.tile([128, BEX], f32, tag=f"tT{c}")
                nc.sync.dma_start(t_[:], tgt[0:BEX, c * 128 : (c + 1) * 128].transpose([1, 0]))
                tT.append(t_)

            wts = []
            for c in range(NH):
                wt = wpool.tile([128, H], f32, tag="w", name=f"wt{c}")
                nc.sync.dma_start(wt[:], wp[c * 128 : (c + 1) * 128, :])
                wts.append(wt)

            bp_sb = cpool.tile([BEX, H], f32, tag="bp_sb")
            v_b = cpool.tile([BEX, H], f32, tag="v_b")
            bv_sb = cpool.tile([BEX, 1], f32, tag="bv_sb")
            for e in range(BEX):
                nc.sync.dma_start(bp_sb[e : e + 1, :], bp[0:1, :])
                nc.sync.dma_start(v_b[e : e + 1, :], vp[0:1, :])
                nc.sync.dma_start(bv_sb[e : e + 1, :], bv[0:1, :])

            zeros = cpool.tile([128, 128], f32, tag="zeros")
            nc.vector.memset(zeros[:], 0.0)
            ebias = cpool.tile([128, 1], f32, tag="ebias")
            nc.vector.memset(ebias[:], EBIAS)
            t_rep32 = []
            t_rep16 = []
            for e in range(BEX):
                r32s, r16s = [], []
                for c in range(NH):
                    r32 = cpool.tile([128, 128], f32, tag=f"t_rep32_{e}_{c}")
                    nc.scalar.activation(
                        r32[:], zeros[:], AF.Identity, bias=tT[c][:, e : e + 1], scale=1.0
                    )
                    r16 = cpool.tile([128, 128], bf16, tag=f"t_rep16_{e}_{c}")
                    nc.vector.tensor_copy(r16[:], r32[:])
                    r32s.append(r32)
                    r16s.append(r16)
                t_rep32.append(r32s)
                t_rep16.append(r16s)

            iota_i = cpool.tile([128, WIN], i32, tag="iota_i")
            nc.gpsimd.iota(iota_i[:], pattern=[[1, WIN]], base=0, channel_multiplier=0)
            iota_f = cpool.tile([128, WIN], f32, tag="iota_f")
            nc.vector.tensor_copy(iota_f[:], iota_i[:])

            # ---------------- phase 0: p = S*sigmoid(v . tanh(W^T t + b)) ----
            ps_hp0 = psB.tile([BEX, 512], f32, tag="hp", name="hp0")
            for c in range(NH):
                nc.tensor.matmul(
                    ps_hp0[:], tT[c][:], wts[c][:, 0:512], start=(c == 0), stop=(c == NH - 1)
                )
            hp_sb = cpool.tile([BEX, H], f32, tag="hp_sb")
            nc.vector.tensor_tensor(hp_sb[:, 0:512], ps_hp0[:], bp_sb[:, 0:512], OP.add)
            ps_hp1 = psB.tile([BEX, 512], f32, tag="hp", name="hp1")
            for c in range(NH):
                nc.tensor.matmul(
                    ps_hp1[:], tT[c][:], wts[c][:, 512:1024], start=(c == 0), stop=(c == NH - 1)
                )
            nc.vector.tensor_tensor(hp_sb[:, 512:1024], ps_hp1[:], bp_sb[:, 512:1024], OP.add)

            nc.scalar.activation(hp_sb[:], hp_sb[:], AF.Tanh)
            ttr_scr = cpool.tile([BEX, H], f32, tag="ttr_scr")
            pre = cpool.tile([BEX, 1], f32, tag="pre")
            nc.vector.tensor_tensor(ttr_scr[:], hp_sb[:], v_b[:], OP.mult)
            nc.vector.tensor_reduce(pre[:], ttr_scr[:], AX.X, OP.add)
            pv = cpool.tile([BEX, 1], f32, tag="pv")
            nc.scalar.activation(pv[:], pre[:], AF.Sigmoid, bias=bv_sb[:], scale=1.0)
            nc.vector.tensor_scalar(pv[:], pv[:], float(S), None, OP.mult)

            s0f = cpool.tile([BEX, 1], f32, tag="s0f")
            nc.vector.tensor_scalar(s0f[:], pv[:], float(WIN // 2), None, OP.subtract)
            nc.vector.tensor_scalar(s0f[:], s0f[:], 0.0, S0MAX, OP.max, OP.min)
            s0i = cpool.tile([BEX, 1], i32, tag="s0i")
            nc.vector.tensor_copy(s0i[:], s0f[:])
            s0ff = cpool.tile([BEX, 1], f32, tag="s0ff")
            nc.vector.tensor_copy(s0ff[:], s0i[:])

            spd = cpool.tile([BEX, 1], f32, tag="spd")
            nc.vector.tensor_tensor(spd[:], s0ff[:], pv[:], OP.subtract)
            nc.sync.dma_start(scr_sp[:], spd[:])

            s0_regs = []
            for e in range(BEX):
                s0_regs.append(
                    nc.values_load(
                        s0i[e : e + 1, 0:1],
                        engines=[ET.SP],
                        min_val=0,
                        max_val=int(S0MAX),
                        skip_runtime_bounds_check=True,
                    )
                )

            # gaussian window factors per example: exp(-(s0 + f - p)^2/(2 s^2))
            gauss = []
            for e in range(BEX):
                sp_b = cpool.tile([128, 1], f32, tag=f"sp_b{e}")
                nc.sync.dma_start(sp_b[:], scr_sp[e : e + 1, 0:1].to_broadcast((128, 1)))
                d = mpool.tile([128, WIN], f32, tag="d", name=f"d_{e}")
                nc.vector.tensor_scalar(d[:], iota_f[:], sp_b[:], None, OP.add)
                nc.scalar.activation(d[:], d[:], AF.Square)
                g = cpool.tile([128, WIN], f32, tag=f"gauss{e}")
                nc.scalar.activation(g[:], d[:], AF.Exp, scale=GEXP)
                gauss.append(g)

            # ---------------- per-example phases -----------------------------
            def scores_phase(e):
                ps = []
                for k in range(NSB):
                    tag = "hp" if k == NSB - 1 else f"sc{k}"
                    ps.append(psB.tile([128, 512], f32, tag=tag, name=f"sc{k}_{e}"))
                for c in range(NH):
                    big = spool.tile([128, S], bf16, tag="stream", name=f"big_{e}_{c}")
                    nc.sync.dma_start(big[:], srcTb[e, c * 128 : (c + 1) * 128, :])
                    for k in range(NSB):
                        nc.tensor.matmul(
                            ps[k][:],
                            t_rep16[e][c][:],
                            big[:, k * 512 : (k + 1) * 512],
                            start=(c == 0),
                            stop=(c == NH - 1),
                        )
                return ps

            def stats_phase(e, ps):
                # softmax denominator over full S (constant shift, no max)
                sums8 = mpool.tile([128, NSB], f32, tag="sums8", name=f"sums8_{e}")
                for k in range(NSB):
                    ej = mpool.tile([128, 512], f32, tag="expjunk", name=f"ej_{e}_{k}")
                    nc.scalar.activation(
                        ej[:],
                        ps[k][:],
                        AF.Exp,
                        bias=ebias[:],
                        scale=SCALE,
                        accum_out=sums8[:, k : k + 1],
                    )
                z = mpool.tile([128, 1], f32, tag="z", name=f"z_{e}")
                nc.vector.tensor_reduce(z[:], sums8[:], AX.X, OP.add)
                rz = mpool.tile([128, 1], f32, tag="rz", name=f"rz_{e}", bufs=2)
                nc.vector.reciprocal(rz[:], z[:])
                return rz

            def window_phase(e, s0_reg, gauss_e, rz):
                # fp32 window: re-fetch window columns, recompute their scores
                # in fp32 (constant shift cancels against Z), then context.
                wins = []
                psw = psB.tile([128, WIN], f32, tag="sc0", name=f"win_ps_{e}")
                for c in range(NH):
                    win = winpool.tile([128, WIN], f32, tag="win", name=f"win_{e}_{c}")
                    nc.sync.dma_start(
                        win[:], srcT[e, c * 128 : (c + 1) * 128, ds(s0_reg, WIN)]
                    )
                    wins.append(win)
                    nc.tensor.matmul(
                        psw[:], t_rep32[e][c][:], win[:], start=(c == 0), stop=(c == NH - 1)
                    )

                expw = mpool.tile([128, WIN], f32, tag="expw", name=f"expw_{e}")
                nc.scalar.activation(expw[:], psw[:], AF.Exp, bias=ebias[:], scale=SCALE)
                attnw = mpool.tile([128, WIN], f32, tag="attnw", name=f"attnw_{e}")
                nc.vector.tensor_tensor(attnw[:], expw[:], gauss_e[:], OP.mult)

                ctx = mpool.tile([128, NH], f32, tag="ctx", name=f"ctx_{e}")
                for c in range(NH):
                    scr = mpool.tile(
                        [128, WIN], f32, tag="scr512", name=f"scr_{e}_{c}", bufs=4
                    )
                    if c % 2 == 0:
                        nc.vector.tensor_tensor(scr[:], wins[c][:], attnw[:], OP.mult)
                        ejc = mpool.tile(
                            [128, WIN], f32, tag="ctxjunk", name=f"cj_{e}_{c}", bufs=2
                        )
                        nc.scalar.activation(
                            ejc[:], scr[:], AF.Identity, accum_out=ctx[:, c : c + 1]
                        )
                    else:
                        nc.gpsimd.tensor_tensor(scr[:], wins[c][:], attnw[:], OP.mult)
                        nc.vector.tensor_reduce(ctx[:, c : c + 1], scr[:], AX.X, OP.add)
                nc.vector.tensor_scalar(ctx[:], ctx[:], rz[:], None, OP.mult)
                nc.sync.dma_start(out[e].transpose([1, 0]), ctx[:])

            ps0 = scores_phase(0)
            rz0 = stats_phase(0, ps0)
            window_phase(0, s0_regs[0], gauss[0], rz0)
            ps1 = scores_phase(1)
            rz1 = stats_phase(1, ps1)
            window_phase(1, s0_regs[1], gauss[1], rz1)

    nc.compile()
    return nc


def _get_nc():
    if "nc" not in _CACHE:
        _CACHE["nc"] = _build()
    return _CACHE["nc"]


def _make_in_maps(src, tgt, wp, bp, vp, bv):
    import ml_dtypes

    srcT = np.ascontiguousarray(src.transpose(0, 2, 1))  # [B, H, S]
    srcTb = srcT.astype(ml_dtypes.bfloat16)
    in_maps = []
    for k in range(N_CORES):
        lo, hi = k * BEX, (k + 1) * BEX
        in_maps.append(
            {
                "srcT": srcT[lo:hi],
                "srcTb": srcTb[lo:hi],
                "tgt": np.ascontiguousarray(tgt[lo:hi]),
                "wp": wp,
                "vp": vp,
                "bp": bp,
                "bv": bv,
            }
        )
    return in_maps


def kernel(source_hidden_states, target_hidden_state, W_p, b_p, v_p, b_v):
    from concourse.bass_utils import run_bass_kernel_spmd

    src = np.asarray(source_hidden_states, dtype=np.float32)
    tgt = np.asarray(target_hidden_state, dtype=np.float32)
    wp = np.asarray(W_p, dtype=np.float32)
    bp = np.asarray(b_p, dtype=np.float32).reshape(1, H)
    vp = np.asarray(v_p, dtype=np.float32).reshape(1, H)
    bv = np.asarray(b_v, dtype=np.float32).reshape(1, 1)

    nc = _get_nc()
    in_maps = _make_in_maps(src, tgt, wp, bp, vp, bv)
    r = run_bass_kernel_spmd(nc, in_maps, list(range(N_CORES)))
    outs = [r.results[k]["out"].reshape(BEX, H) for k in range(N_CORES)]
    return np.concatenate(outs, axis=0)


# revision 16
# speedup vs baseline: 1.6716x; 1.0005x over previous
"""Luong local-p attention (scaled-dot, gaussian window) on 8 trn2 cores.

Strategy (data-parallel over batch, 2 examples/core):
  - Host: transpose source_hidden_states to [H, S] per example so the score
    matmul can contract over H on the TensorEngine with the target vector
    replicated as the stationary operand (scores come out replicated across
    all 128 partitions, which is exactly the layout the windowed context
    multiply needs). Ships a bf16 copy (streamed once for scores/softmax
    denominator) and keeps the fp32 copy for the window re-read.
  - Device per example:
      p = S*sigmoid(v_p . tanh(W_p^T t + b_p) + b_v)   (fp32 PE matmul + ACT)
      scores[s] = (src[s,:] . t)/sqrt(H)                (bf16 PE, psum-acc)
      softmax denominator Z over full S with a CONSTANT shift of -8 instead
      of the max (scores are ~N(0,1); fp32 range makes a computed max
      unnecessary, and the constant shift cancels exactly in the ratio)
      window [s0, s0+320), s0 = clamp(floor(p)-160, 0, S-320) covers every
      position whose gaussian factor exceeds ~3.7e-6 (5 sigma); window
      scores are recomputed in fp32 from the re-fetched fp32 window columns
      so the attention weights that matter are fp32-accurate. Context =
      windowed multiply-reduce spread across GPSIMD/DVE (multiplies) and
      ACT/DVE (free-dim reductions).
  - Scheduling: the p-computation's 16 fp32 matmuls are interleaved into the
    DMA-gaps of example 0's score groups (2 psum banks), score blocks 6 and
    7 run as late sweeps over the still-resident stream tiles once those
    banks free, and the dynamic-window register only loads on the SP engine.
"""

import numpy as np

N_CORES = 8
B, S, H = 16, 4096, 1024
BEX = B // N_CORES  # examples per core
NH = H // 128  # h-chunks of 128 partitions
NSB = S // 512  # s-blocks of 512
WIN = 320
SCALE = 1.0 / 32.0  # 1/sqrt(H)
GEXP = -1.0 / 2048.0  # -1/(2*sigma^2), sigma = WINDOW/2 = 32
EBIAS = -8.0  # constant softmax shift
S0MAX = float(S - WIN)

_CACHE = {}


def _build():
    import concourse.bacc as bacc
    import concourse.bass as bass
    import concourse.mybir as mybir
    import concourse.tile as tile

    f32 = mybir.dt.float32
    bf16 = mybir.dt.bfloat16
    i32 = mybir.dt.int32
    AF = mybir.ActivationFunctionType
    OP = mybir.AluOpType
    AX = mybir.AxisListType
    ET = mybir.EngineType
    ds = bass.ds

    nc = bacc.Bacc("TRN2", target_bir_lowering=False, debug=False, num_devices=N_CORES)
    srcT = nc.dram_tensor("srcT", [BEX, H, S], f32, kind="ExternalInput").ap()
    srcTb = nc.dram_tensor("srcTb", [BEX, H, S], bf16, kind="ExternalInput").ap()
    tgtT = nc.dram_tensor("tgtT", [NH, 128, BEX], f32, kind="ExternalInput").ap()
    wp = nc.dram_tensor("wp", [H, H], f32, kind="ExternalInput").ap()
    vp = nc.dram_tensor("vp", [1, H], f32, kind="ExternalInput").ap()
    bp = nc.dram_tensor("bp", [1, H], f32, kind="ExternalInput").ap()
    bv = nc.dram_tensor("bv", [1, 1], f32, kind="ExternalInput").ap()
    out = nc.dram_tensor("out", [BEX, NH, 128], f32, kind="ExternalOutput").ap()
    scr_sp = nc.dram_tensor("scr_sp", [BEX, 1], f32).ap()

    with tile.TileContext(nc) as tc:
        with (
            tc.tile_pool(name="cpool", bufs=1) as cpool,
            tc.tile_pool(name="wpool", bufs=8) as wpool,
            tc.tile_pool(name="spool", bufs=10) as spool,
            tc.tile_pool(name="winpool", bufs=10) as winpool,
            tc.tile_pool(name="mpool", bufs=2) as mpool,
            tc.tile_pool(name="psB", bufs=1, space="PSUM") as psB,
        ):
            # ---------------- setup: tiny DMAs + stationary operands ---------
            tT = []
            for c in range(NH):
                t_ = cpool.tile([128, BEX], f32, tag=f"tT{c}")
                nc.sync.dma_start(t_[:], tgtT[c])
                tT.append(t_)

            wts = []
            for c in range(NH):
                wt = wpool.tile([128, H], f32, tag="w", name=f"wt{c}")
                nc.sync.dma_start(wt[:], wp[c * 128 : (c + 1) * 128, :])
                wts.append(wt)

            bp_sb = cpool.tile([BEX, H], f32, tag="bp_sb")
            v_b = cpool.tile([BEX, H], f32, tag="v_b")
            bv_sb = cpool.tile([BEX, 1], f32, tag="bv_sb")
            for e in range(BEX):
                nc.sync.dma_start(bp_sb[e : e + 1, :], bp[0:1, :])
                nc.sync.dma_start(v_b[e : e + 1, :], vp[0:1, :])
                nc.sync.dma_start(bv_sb[e : e + 1, :], bv[0:1, :])

            zeros = cpool.tile([128, 128], f32, tag="zeros")
            nc.vector.memset(zeros[:], 0.0)
            ebias = cpool.tile([128, 1], f32, tag="ebias")
            nc.vector.memset(ebias[:], EBIAS)
            t_rep32 = []
            t_rep16 = []
            for e in range(BEX):
                r32s, r16s = [], []
                for c in range(NH):
                    r32 = cpool.tile([128, 128], f32, tag=f"t_rep32_{e}_{c}")
                    nc.scalar.activation(
                        r32[:], zeros[:], AF.Identity, bias=tT[c][:, e : e + 1], scale=1.0
                    )
                    r16 = cpool.tile([128, 128], bf16, tag=f"t_rep16_{e}_{c}")
                    nc.vector.tensor_copy(r16[:], r32[:])
                    r32s.append(r32)
                    r16s.append(r16)
                t_rep32.append(r32s)
                t_rep16.append(r16s)

            iota_i = cpool.tile([128, WIN], i32, tag="iota_i")
            nc.gpsimd.iota(iota_i[:], pattern=[[1, WIN]], base=0, channel_multiplier=0)
            iota_f = cpool.tile([128, WIN], f32, tag="iota_f")
            nc.vector.tensor_copy(iota_f[:], iota_i[:])

            # ---------------- ex0 scores interleaved with phase-0 matmuls ----
            ps_hp0 = psB.tile([BEX, 512], f32, tag="hp0", name="hp0")
            ps_hp1 = psB.tile([BEX, 512], f32, tag="hp1", name="hp1")
            ps0 = []
            for k in range(NSB):
                tag = {6: "hp0", 7: "hp1"}.get(k, f"sc{k}")
                ps0.append(psB.tile([128, 512], f32, tag=tag, name=f"sc{k}_0"))

            bigs0 = []
            for c in range(NH):
                big = spool.tile([128, S], bf16, tag="stream", name=f"big_0_{c}")
                nc.sync.dma_start(big[:], srcTb[0, c * 128 : (c + 1) * 128, :])
                bigs0.append(big)
                for k in range(6):
                    nc.tensor.matmul(
                        ps0[k][:],
                        t_rep16[0][c][:],
                        big[:, k * 512 : (k + 1) * 512],
                        start=(c == 0),
                        stop=(c == NH - 1),
                    )
                nc.tensor.matmul(
                    ps_hp0[:], tT[c][:], wts[c][:, 0:512], start=(c == 0), stop=(c == NH - 1)
                )
                nc.tensor.matmul(
                    ps_hp1[:], tT[c][:], wts[c][:, 512:1024], start=(c == 0), stop=(c == NH - 1)
                )

            # ---------------- phase 0 tail: p, s0, gaussian ------------------
            hp_sb = cpool.tile([BEX, H], f32, tag="hp_sb")
            nc.vector.tensor_tensor(hp_sb[:, 0:512], ps_hp0[:], bp_sb[:, 0:512], OP.add)
            nc.vector.tensor_tensor(hp_sb[:, 512:1024], ps_hp1[:], bp_sb[:, 512:1024], OP.add)
            nc.scalar.activation(hp_sb[:], hp_sb[:], AF.Tanh)
            ttr_scr = cpool.tile([BEX, H], f32, tag="ttr_scr")
            pre = cpool.tile([BEX, 1], f32, tag="pre")
            nc.vector.tensor_tensor(ttr_scr[:], hp_sb[:], v_b[:], OP.mult)
            nc.vector.tensor_reduce(pre[:], ttr_scr[:], AX.X, OP.add)
            pv = cpool.tile([BEX, 1], f32, tag="pv")
            nc.scalar.activation(pv[:], pre[:], AF.Sigmoid, bias=bv_sb[:], scale=1.0)
            nc.vector.tensor_scalar(pv[:], pv[:], float(S), None, OP.mult)

            s0f = cpool.tile([BEX, 1], f32, tag="s0f")
            nc.vector.tensor_scalar(s0f[:], pv[:], float(WIN // 2), None, OP.subtract)
            nc.vector.tensor_scalar(s0f[:], s0f[:], 0.0, S0MAX, OP.max, OP.min)
            s0i = cpool.tile([BEX, 1], i32, tag="s0i")
            nc.vector.tensor_copy(s0i[:], s0f[:])
            s0ff = cpool.tile([BEX, 1], f32, tag="s0ff")
            nc.vector.tensor_copy(s0ff[:], s0i[:])

            spd = cpool.tile([BEX, 1], f32, tag="spd")
            nc.vector.tensor_tensor(spd[:], s0ff[:], pv[:], OP.subtract)
            nc.sync.dma_start(scr_sp[:], spd[:])

            s0_regs = []
            for e in range(BEX):
                s0_regs.append(
                    nc.values_load(
                        s0i[e : e + 1, 0:1],
                        engines=[ET.SP],
                        min_val=0,
                        max_val=int(S0MAX),
                        skip_runtime_bounds_check=True,
                    )
                )

            gauss = []
            for e in range(BEX):
                sp_b = cpool.tile([128, 1], f32, tag=f"sp_b{e}")
                nc.sync.dma_start(sp_b[:], scr_sp[e : e + 1, 0:1].to_broadcast((128, 1)))
                d = mpool.tile([128, WIN], f32, tag="d", name=f"d_{e}")
                nc.vector.tensor_scalar(d[:], iota_f[:], sp_b[:], None, OP.add)
                nc.scalar.activation(d[:], d[:], AF.Square)
                g = cpool.tile([128, WIN], f32, tag=f"gauss{e}")
                nc.scalar.activation(g[:], d[:], AF.Exp, scale=GEXP)
                gauss.append(g)

            # ---------------- ex0 late k6/k7 sweeps (banks just freed) -------
            for k in (6, 7):
                for c in range(NH):
                    nc.tensor.matmul(
                        ps0[k][:],
                        t_rep16[0][c][:],
                        bigs0[c][:, k * 512 : (k + 1) * 512],
                        start=(c == 0),
                        stop=(c == NH - 1),
                    )

            # ---------------- shared phase helpers ---------------------------
            def scores_phase(e):
                ps = []
                for k in range(NSB):
                    tag = {6: "hp0", 7: "hp1"}.get(k, f"sc{k}")
                    ps.append(psB.tile([128, 512], f32, tag=tag, name=f"sc{k}_{e}"))
                for c in range(NH):
                    big = spool.tile([128, S], bf16, tag="stream", name=f"big_{e}_{c}")
                    nc.sync.dma_start(big[:], srcTb[e, c * 128 : (c + 1) * 128, :])
                    for k in range(NSB):
                        nc.tensor.matmul(
                            ps[k][:],
                            t_rep16[e][c][:],
                            big[:, k * 512 : (k + 1) * 512],
                            start=(c == 0),
                            stop=(c == NH - 1),
                        )
                return ps

            def stats_phase(e, ps):
                # softmax denominator over full S (constant shift, no max)
                sums8 = mpool.tile([128, NSB], f32, tag="sums8", name=f"sums8_{e}")
                for k in range(NSB):
                    ej = mpool.tile([128, 512], f32, tag="expjunk", name=f"ej_{e}_{k}")
                    nc.scalar.activation(
                        ej[:],
                        ps[k][:],
                        AF.Exp,
                        bias=ebias[:],
                        scale=SCALE,
                        accum_out=sums8[:, k : k + 1],
                    )
                z = mpool.tile([128, 1], f32, tag="z", name=f"z_{e}")
                nc.vector.tensor_reduce(z[:], sums8[:], AX.X, OP.add)
                rz = mpool.tile([128, 1], f32, tag="rz", name=f"rz_{e}", bufs=2)
                nc.vector.reciprocal(rz[:], z[:])
                return rz

            def window_phase(e, s0_reg, gauss_e, rz):
                # fp32 window: re-fetch window columns, recompute their scores
                # in fp32 (constant shift cancels against Z), then context.
                wins = []
                psw = psB.tile([128, WIN], f32, tag="sc5", name=f"win_ps_{e}")
                for c in range(NH):
                    win = winpool.tile([128, WIN], f32, tag="win", name=f"win_{e}_{c}")
                    nc.sync.dma_start(
                        win[:], srcT[e, c * 128 : (c + 1) * 128, ds(s0_reg, WIN)]
                    )
                    wins.append(win)
                    nc.tensor.matmul(
                        psw[:], t_rep32[e][c][:], win[:], start=(c == 0), stop=(c == NH - 1)
                    )

                expw = mpool.tile([128, WIN], f32, tag="expw", name=f"expw_{e}")
                nc.scalar.activation(expw[:], psw[:], AF.Exp, bias=ebias[:], scale=SCALE)
                attnw = mpool.tile([128, WIN], f32, tag="attnw", name=f"attnw_{e}")
                nc.vector.tensor_tensor(attnw[:], expw[:], gauss_e[:], OP.mult)

                ctx = mpool.tile([128, NH], f32, tag="ctx", name=f"ctx_{e}")
                for c in range(NH):
                    scr = mpool.tile(
                        [128, WIN], f32, tag="scr512", name=f"scr_{e}_{c}", bufs=4
                    )
                    if c % 2 == 0:
                        nc.vector.tensor_tensor(scr[:], wins[c][:], attnw[:], OP.mult)
                        ejc = mpool.tile(
                            [128, WIN], f32, tag="ctxjunk", name=f"cj_{e}_{c}", bufs=2
                        )
                        nc.scalar.activation(
                            ejc[:], scr[:], AF.Identity, accum_out=ctx[:, c : c + 1]
                        )
                    else:
                        nc.gpsimd.tensor_tensor(scr[:], wins[c][:], attnw[:], OP.mult)
                        nc.vector.tensor_reduce(ctx[:, c : c + 1], scr[:], AX.X, OP.add)
                nc.vector.tensor_scalar(ctx[:], ctx[:], rz[:], None, OP.mult)
                nc.sync.dma_start(out[e].transpose([1, 0]), ctx[:])

            rz0 = stats_phase(0, ps0)
            window_phase(0, s0_regs[0], gauss[0], rz0)
            ps1 = scores_phase(1)
            rz1 = stats_phase(1, ps1)
            window_phase(1, s0_regs[1], gauss[1], rz1)

    nc.compile()
    return nc


def _get_nc():
    if "nc" not in _CACHE:
        _CACHE["nc"] = _build()
    return _CACHE["nc"]


def _make_in_maps(src, tgt, wp, bp, vp, bv):
    import ml_dtypes

    srcT = np.ascontiguousarray(src.transpose(0, 2, 1))  # [B, H, S]
    srcTb = srcT.astype(ml_dtypes.bfloat16)
    in_maps = []
    for k in range(N_CORES):
        lo, hi = k * BEX, (k + 1) * BEX
        tgtT = np.ascontiguousarray(
            tgt[lo:hi].reshape(BEX, NH, 128).transpose(1, 2, 0)
        )  # [NH, 128, BEX]
        in_maps.append(
            {
                "srcT": srcT[lo:hi],
                "srcTb": srcTb[lo:hi],
                "tgtT": tgtT,
                "wp": wp,
                "vp": vp,
                "bp": bp,
                "bv": bv,
            }
        )
    return in_maps


def kernel(source_hidden_states, target_hidden_state, W_p, b_p, v_p, b_v):
    from concourse.bass_utils import run_bass_kernel_spmd

    src = np.asarray(source_hidden_states, dtype=np.float32)
    tgt = np.asarray(target_hidden_state, dtype=np.float32)
    wp = np.asarray(W_p, dtype=np.float32)
    bp = np.asarray(b_p, dtype=np.float32).reshape(1, H)
    vp = np.asarray(v_p, dtype=np.float32).reshape(1, H)
    bv = np.asarray(b_v, dtype=np.float32).reshape(1, 1)

    nc = _get_nc()
    in_maps = _make_in_maps(src, tgt, wp, bp, vp, bv)
    r = run_bass_kernel_spmd(nc, in_maps, list(range(N_CORES)))
    outs = [r.results[k]["out"].reshape(BEX, H) for k in range(N_CORES)]
    return np.concatenate(outs, axis=0)


# revision 20
# speedup vs baseline: 1.9780x; 1.1833x over previous
"""Luong local-p attention (scaled-dot, gaussian window) on 8 trn2 cores.

Strategy (data-parallel over batch, 2 examples/core):
  - Host: transpose source_hidden_states to [H, S] per example so the score
    matmul can contract over H on the TensorEngine with the target vector
    replicated as the stationary operand (scores come out replicated across
    the output partitions, which is exactly the layout the windowed context
    multiply needs). Ships a bf16 copy (streamed once for scores/softmax
    denominator) and keeps the fp32 copy for the window re-read.
  - Device per example:
      p = S*sigmoid(v_p . tanh(W_p^T t + b_p) + b_v)   (fp32 PE matmul + ACT)
      scores[s] = (src[s,:] . t)/sqrt(H)                (bf16 PE, psum-acc)
      softmax denominator Z over full S with a CONSTANT shift of -8 instead
      of the max (scores are ~N(0,1); fp32 range makes a computed max
      unnecessary, and the constant shift cancels exactly in the ratio)
      window [s0, s0+320), s0 = clamp(floor(p)-160, 0, S-320) covers every
      position whose gaussian factor exceeds ~3.7e-6 (5 sigma); window
      scores are recomputed in fp32 from the re-fetched fp32 window columns
      so the attention weights that matter are fp32-accurate. Context =
      windowed multiply-reduce spread across GPSIMD/DVE (multiplies) and
      ACT/DVE (free-dim reductions).
  - Resources: score psum uses 4 banks (two s-blocks per bank, 64-row
    replication via PE column tiling), the p-computation owns 2 banks and
    its 16 fp32 matmuls interleave into the DMA-gaps of example 0's score
    groups, the window recompute owns its own double-buffered bank. Window
    and output DMAs dispatch from the Activation queue to unload SP.
"""

import numpy as np

N_CORES = 8
B, S, H = 16, 4096, 1024
BEX = B // N_CORES  # examples per core
NH = H // 128  # h-chunks of 128 partitions
NSB = S // 512  # s-blocks of 512
NBK = NSB // 2  # psum banks for scores (2 blocks per bank)
WIN = 320
SCALE = 1.0 / 32.0  # 1/sqrt(H)
GEXP = -1.0 / 2048.0  # -1/(2*sigma^2), sigma = WINDOW/2 = 32
EBIAS = -8.0  # constant softmax shift
S0MAX = float(S - WIN)

_CACHE = {}


def _build():
    import concourse.bacc as bacc
    import concourse.bass as bass
    import concourse.mybir as mybir
    import concourse.tile as tile

    f32 = mybir.dt.float32
    bf16 = mybir.dt.bfloat16
    i32 = mybir.dt.int32
    AF = mybir.ActivationFunctionType
    OP = mybir.AluOpType
    AX = mybir.AxisListType
    ET = mybir.EngineType
    ds = bass.ds

    nc = bacc.Bacc("TRN2", target_bir_lowering=False, debug=False, num_devices=N_CORES)
    srcT = nc.dram_tensor("srcT", [BEX, H, S], f32, kind="ExternalInput").ap()
    srcTb = nc.dram_tensor("srcTb", [BEX, H, S], bf16, kind="ExternalInput").ap()
    tgtT = nc.dram_tensor("tgtT", [NH, 128, BEX], f32, kind="ExternalInput").ap()
    wp = nc.dram_tensor("wp", [H, H], f32, kind="ExternalInput").ap()
    vp = nc.dram_tensor("vp", [1, H], f32, kind="ExternalInput").ap()
    bp = nc.dram_tensor("bp", [1, H], f32, kind="ExternalInput").ap()
    bv = nc.dram_tensor("bv", [1, 1], f32, kind="ExternalInput").ap()
    out = nc.dram_tensor("out", [BEX, 128, NH], f32, kind="ExternalOutput").ap()
    scr_sp = nc.dram_tensor("scr_sp", [BEX, 1], f32).ap()

    with tile.TileContext(nc) as tc:
        with (
            tc.tile_pool(name="cpool", bufs=1) as cpool,
            tc.tile_pool(name="spool", bufs=5) as spool,
            tc.tile_pool(name="winpool", bufs=10) as winpool,
            tc.tile_pool(name="mpool", bufs=2) as mpool,
            tc.tile_pool(name="psB", bufs=1, space="PSUM") as psB,
        ):
            # ---------------- setup: batched small DMAs ----------------------
            tTall = cpool.tile([128, NH, BEX], f32, tag="tTall")
            nc.sync.dma_start(tTall[:], tgtT.rearrange("c p e -> p c e"))
            tT = [tTall[:, c, :] for c in range(NH)]

            wtall = cpool.tile([128, NH, H], f32, tag="wtall")
            nc.scalar.dma_start(
                wtall[:], wp.rearrange("(c p) j -> p c j", p=128)
            )
            wts = [wtall[:, c, :] for c in range(NH)]

            bp_sb = cpool.tile([BEX, H], f32, tag="bp_sb")
            v_b = cpool.tile([BEX, H], f32, tag="v_b")
            bv_sb = cpool.tile([BEX, 1], f32, tag="bv_sb")
            for e in range(BEX):
                nc.sync.dma_start(bp_sb[e : e + 1, :], bp[0:1, :])
                nc.sync.dma_start(v_b[e : e + 1, :], vp[0:1, :])
                nc.sync.dma_start(bv_sb[e : e + 1, :], bv[0:1, :])

            zeros = cpool.tile([128, 128], f32, tag="zeros")
            nc.vector.memset(zeros[:], 0.0)
            ebias = cpool.tile([128, 1], f32, tag="ebias")
            nc.vector.memset(ebias[:], EBIAS)
            t_rep32 = []
            t_rep16 = []
            for e in range(BEX):
                r32s, r16s = [], []
                for c in range(NH):
                    r32 = cpool.tile([128, 128], f32, tag=f"t_rep32_{e}_{c}")
                    nc.scalar.activation(
                        r32[:], zeros[:], AF.Identity, bias=tT[c][:, e : e + 1], scale=1.0
                    )
                    r16 = cpool.tile([128, 128], bf16, tag=f"t_rep16_{e}_{c}")
                    nc.vector.tensor_copy(r16[:], r32[:])
                    r32s.append(r32)
                    r16s.append(r16)
                t_rep32.append(r32s)
                t_rep16.append(r16s)

            iota_i = cpool.tile([128, WIN], i32, tag="iota_i")
            nc.gpsimd.iota(iota_i[:], pattern=[[1, WIN]], base=0, channel_multiplier=0)
            iota_f = cpool.tile([128, WIN], f32, tag="iota_f")
            nc.vector.tensor_copy(iota_f[:], iota_i[:])

            def emit_score_mms(e, ps, c, big):
                for k in range(NSB):
                    j, half = divmod(k, 2)
                    pslice = ps[j][64 * half : 64 * (half + 1), :]
                    nc.tensor.matmul(
                        pslice,
                        t_rep16[e][c][:, 0:64],
                        big[:, k * 512 : (k + 1) * 512],
                        start=(c == 0),
                        stop=(c == NH - 1),
                        tile_position=(0, 64 * half),
                        skip_group_check=True,
                    )

            # ---------------- ex0 scores interleaved with phase-0 matmuls ----
            ps_hp0 = psB.tile([BEX, 512], f32, tag="hp0", name="hp0")
            ps_hp1 = psB.tile([BEX, 512], f32, tag="hp1", name="hp1")
            ps0 = [
                psB.tile([128, 512], f32, tag=f"scA{j}", name=f"scA{j}_0")
                for j in range(NBK)
            ]
            for c in range(NH):
                big = spool.tile([128, S], bf16, tag="stream", name=f"big_0_{c}")
                nc.sync.dma_start(big[:], srcTb[0, c * 128 : (c + 1) * 128, :])
                emit_score_mms(0, ps0, c, big)
                nc.tensor.matmul(
                    ps_hp0[:], tT[c][:], wts[c][:, 0:512], start=(c == 0), stop=(c == NH - 1)
                )
                nc.tensor.matmul(
                    ps_hp1[:], tT[c][:], wts[c][:, 512:1024], start=(c == 0), stop=(c == NH - 1)
                )

            # ---------------- phase 0 tail: p, s0, gaussian ------------------
            hp_sb = cpool.tile([BEX, H], f32, tag="hp_sb")
            nc.vector.tensor_tensor(hp_sb[:, 0:512], ps_hp0[:], bp_sb[:, 0:512], OP.add)
            nc.vector.tensor_tensor(hp_sb[:, 512:1024], ps_hp1[:], bp_sb[:, 512:1024], OP.add)
            nc.scalar.activation(hp_sb[:], hp_sb[:], AF.Tanh)
            ttr_scr = cpool.tile([BEX, H], f32, tag="ttr_scr")
            pre = cpool.tile([BEX, 1], f32, tag="pre")
            nc.vector.tensor_tensor(ttr_scr[:], hp_sb[:], v_b[:], OP.mult)
            nc.vector.tensor_reduce(pre[:], ttr_scr[:], AX.X, OP.add)
            pv = cpool.tile([BEX, 1], f32, tag="pv")
            nc.scalar.activation(pv[:], pre[:], AF.Sigmoid, bias=bv_sb[:], scale=1.0)
            nc.vector.tensor_scalar(pv[:], pv[:], float(S), None, OP.mult)

            s0f = cpool.tile([BEX, 1], f32, tag="s0f")
            nc.vector.tensor_scalar(s0f[:], pv[:], float(WIN // 2), None, OP.subtract)
            nc.vector.tensor_scalar(s0f[:], s0f[:], 0.0, S0MAX, OP.max, OP.min)
            s0i = cpool.tile([BEX, 1], i32, tag="s0i")
            nc.vector.tensor_copy(s0i[:], s0f[:])
            s0ff = cpool.tile([BEX, 1], f32, tag="s0ff")
            nc.vector.tensor_copy(s0ff[:], s0i[:])

            spd = cpool.tile([BEX, 1], f32, tag="spd")
            nc.vector.tensor_tensor(spd[:], s0ff[:], pv[:], OP.subtract)
            nc.sync.dma_start(scr_sp[:], spd[:])

            s0_regs = []
            for e in range(BEX):
                s0_regs.append(
                    nc.values_load(
                        s0i[e : e + 1, 0:1],
                        engines=[ET.SP, ET.Activation],
                        min_val=0,
                        max_val=int(S0MAX),
                        skip_runtime_bounds_check=True,
                    )
                )

            gauss = []
            for e in range(BEX):
                sp_b = cpool.tile([128, 1], f32, tag=f"sp_b{e}")
                nc.sync.dma_start(sp_b[:], scr_sp[e : e + 1, 0:1].to_broadcast((128, 1)))
                d = mpool.tile([128, WIN], f32, tag="d", name=f"d_{e}")
                nc.vector.tensor_scalar(d[:], iota_f[:], sp_b[:], None, OP.add)
                nc.scalar.activation(d[:], d[:], AF.Square)
                g = cpool.tile([128, WIN], f32, tag=f"gauss{e}")
                nc.scalar.activation(g[:], d[:], AF.Exp, scale=GEXP)
                gauss.append(g)

            # ---------------- shared phase helpers ---------------------------
            def scores_phase(e):
                ps = [
                    psB.tile([128, 512], f32, tag=f"scA{j}", name=f"scA{j}_{e}")
                    for j in range(NBK)
                ]
                for c in range(NH):
                    big = spool.tile([128, S], bf16, tag="stream", name=f"big_{e}_{c}")
                    nc.sync.dma_start(big[:], srcTb[e, c * 128 : (c + 1) * 128, :])
                    emit_score_mms(e, ps, c, big)
                return ps

            def stats_phase(e, ps):
                # softmax denominator over full S (constant shift, no max);
                # column j of sums4 holds block 2j sums in rows 0:64 and
                # block 2j+1 sums in rows 64:128.
                sums4 = mpool.tile([128, NBK], f32, tag="sums4", name=f"sums4_{e}")
                for j in range(NBK):
                    for half in range(2):
                        lo, hi = 64 * half, 64 * (half + 1)
                        ej = mpool.tile([128, 512], f32, tag="expjunk", name=f"ej_{e}_{j}_{half}")
                        nc.scalar.activation(
                            ej[lo:hi, :],
                            ps[j][lo:hi, :],
                            AF.Exp,
                            bias=ebias[lo:hi, :],
                            scale=SCALE,
                            accum_out=sums4[lo:hi, j : j + 1],
                        )
                z4 = mpool.tile([128, 1], f32, tag="z4", name=f"z4_{e}")
                nc.vector.tensor_reduce(z4[:], sums4[:], AX.X, OP.add)
                zsw = mpool.tile([128, 1], f32, tag="zsw", name=f"zsw_{e}")
                nc.sync.dma_start(zsw[0:64, :], z4[64:128, :])
                nc.sync.dma_start(zsw[64:128, :], z4[0:64, :])
                zf = mpool.tile([128, 1], f32, tag="zf", name=f"zf_{e}")
                nc.vector.tensor_tensor(zf[:], z4[:], zsw[:], OP.add)
                rz = mpool.tile([128, 1], f32, tag="rz", name=f"rz_{e}", bufs=2)
                nc.vector.reciprocal(rz[:], zf[:])
                return rz

            def window_phase(e, s0_reg, gauss_e, rz):
                # fp32 window: re-fetch window columns, recompute their scores
                # in fp32 (constant shift cancels against Z), then context.
                wins = []
                psw = psB.tile([128, WIN], f32, tag="psw", name=f"win_ps_{e}", bufs=2)
                for cc in range(NH // 2):
                    winp = winpool.tile(
                        [128, 2, WIN], f32, tag="win", name=f"win_{e}_{cc}", bufs=5
                    )
                    nc.scalar.dma_start(
                        winp[:],
                        srcT[e, 256 * cc : 256 * (cc + 1), ds(s0_reg, WIN)].rearrange(
                            "(c p) w -> p c w", p=128
                        ),
                    )
                    wins.extend([winp[:, 0, :], winp[:, 1, :]])
                for c in range(NH):
                    nc.tensor.matmul(
                        psw[:],
                        t_rep32[e][c][:],
                        wins[c],
                        start=(c == 0),
                        stop=(c == NH - 1),
                    )

                expw = mpool.tile([128, WIN], f32, tag="expw", name=f"expw_{e}")
                nc.scalar.activation(expw[:], psw[:], AF.Exp, bias=ebias[:], scale=SCALE)
                attnw = mpool.tile([128, WIN], f32, tag="attnw", name=f"attnw_{e}")
                nc.vector.tensor_tensor(attnw[:], expw[:], gauss_e[:], OP.mult)

                ctx = mpool.tile([128, NH], f32, tag="ctx", name=f"ctx_{e}")
                for c in range(NH):
                    scr = mpool.tile(
                        [128, WIN], f32, tag="scr512", name=f"scr_{e}_{c}", bufs=4
                    )
                    if c % 2 == 0:
                        nc.vector.tensor_tensor(scr[:], wins[c], attnw[:], OP.mult)
                        ejc = mpool.tile(
                            [128, WIN], f32, tag="ctxjunk", name=f"cj_{e}_{c}", bufs=2
                        )
                        nc.scalar.activation(
                            ejc[:], scr[:], AF.Identity, accum_out=ctx[:, c : c + 1]
                        )
                    else:
                        nc.gpsimd.tensor_tensor(scr[:], wins[c], attnw[:], OP.mult)
                        nc.vector.tensor_reduce(ctx[:, c : c + 1], scr[:], AX.X, OP.add)
                nc.vector.tensor_scalar(ctx[:], ctx[:], rz[:], None, OP.mult)
                nc.scalar.dma_start(out[e], ctx[:])

            rz0 = stats_phase(0, ps0)
            window_phase(0, s0_regs[0], gauss[0], rz0)
            ps1 = scores_phase(1)
            rz1 = stats_phase(1, ps1)
            window_phase(1, s0_regs[1], gauss[1], rz1)

    nc.compile()
    return nc


def _get_nc():
    if "nc" not in _CACHE:
        _CACHE["nc"] = _build()
    return _CACHE["nc"]


def _make_in_maps(src, tgt, wp, bp, vp, bv):
    import ml_dtypes

    srcT = np.ascontiguousarray(src.transpose(0, 2, 1))  # [B, H, S]
    srcTb = srcT.astype(ml_dtypes.bfloat16)
    in_maps = []
    for k in range(N_CORES):
        lo, hi = k * BEX, (k + 1) * BEX
        tgtT = np.ascontiguousarray(
            tgt[lo:hi].reshape(BEX, NH, 128).transpose(1, 2, 0)
        )  # [NH, 128, BEX]
        in_maps.append(
            {
                "srcT": srcT[lo:hi],
                "srcTb": srcTb[lo:hi],
                "tgtT": tgtT,
                "wp": wp,
                "vp": vp,
                "bp": bp,
                "bv": bv,
            }
        )
    return in_maps


def kernel(source_hidden_states, target_hidden_state, W_p, b_p, v_p, b_v):
    from concourse.bass_utils import run_bass_kernel_spmd

    src = np.asarray(source_hidden_states, dtype=np.float32)
    tgt = np.asarray(target_hidden_state, dtype=np.float32)
    wp = np.asarray(W_p, dtype=np.float32)
    bp = np.asarray(b_p, dtype=np.float32).reshape(1, H)
    vp = np.asarray(v_p, dtype=np.float32).reshape(1, H)
    bv = np.asarray(b_v, dtype=np.float32).reshape(1, 1)

    nc = _get_nc()
    in_maps = _make_in_maps(src, tgt, wp, bp, vp, bv)
    r = run_bass_kernel_spmd(nc, in_maps, list(range(N_CORES)))
    # out[e] is ctx [128, NH]; context[b, h] with h = c*128 + p lives at
    # out[b, p, c] -> transpose to [NH, 128] then flatten.
    outs = [
        r.results[k]["out"].transpose(0, 2, 1).reshape(BEX, H) for k in range(N_CORES)
    ]
    return np.concatenate(outs, axis=0)


# revision 21
# speedup vs baseline: 2.1484x; 1.0861x over previous
"""Luong local-p attention (scaled-dot, gaussian window) on 8 trn2 cores.

Strategy (data-parallel over batch, 2 examples/core):
  - Host: transpose source_hidden_states to [H, S] per example so the score
    matmul can contract over H on the TensorEngine with the target vector
    replicated as the stationary operand (scores come out replicated across
    the output partitions, which is exactly the layout the windowed context
    multiply needs). Ships a bf16 copy (streamed once for scores/softmax
    denominator) and keeps the fp32 copy for the window re-read.
  - Device per example:
      p = S*sigmoid(v_p . tanh(W_p^T t + b_p) + b_v)   (fp32 PE matmul + ACT)
      scores[s] = (src[s,:] . t)/sqrt(H)                (bf16 PE, psum-acc)
      softmax denominator Z over full S with a CONSTANT shift of -8 instead
      of the max (scores are ~N(0,1); fp32 range makes a computed max
      unnecessary, and the constant shift cancels exactly in the ratio)
      window [s0, s0+320), s0 = clamp(floor(p)-160, 0, S-320) covers every
      position whose gaussian factor exceeds ~3.7e-6 (5 sigma); window
      scores are recomputed in fp32 from the re-fetched fp32 window columns
      so the attention weights that matter are fp32-accurate. Context =
      windowed multiply-reduce spread across GPSIMD/DVE (multiplies) and
      ACT/DVE (free-dim reductions).
  - Resources: score psum uses 4 banks (two s-blocks per bank, 64-row
    replication via PE column tiling), the p-computation owns 2 banks and
    its 16 fp32 matmuls interleave into the DMA-gaps of example 0's score
    groups, the window recompute owns its own double-buffered bank. Window
    and output DMAs dispatch from the Activation queue to unload SP.
"""

import numpy as np

N_CORES = 8
B, S, H = 16, 4096, 1024
BEX = B // N_CORES  # examples per core
NH = H // 128  # h-chunks of 128 partitions
NSB = S // 512  # s-blocks of 512
NBK = NSB // 2  # psum banks for scores (2 blocks per bank)
WIN = 320
SCALE = 1.0 / 32.0  # 1/sqrt(H)
GEXP = -1.0 / 2048.0  # -1/(2*sigma^2), sigma = WINDOW/2 = 32
EBIAS = -8.0  # constant softmax shift
S0MAX = float(S - WIN)

_CACHE = {}


def _build():
    import concourse.bacc as bacc
    import concourse.bass as bass
    import concourse.mybir as mybir
    import concourse.tile as tile

    f32 = mybir.dt.float32
    bf16 = mybir.dt.bfloat16
    i32 = mybir.dt.int32
    AF = mybir.ActivationFunctionType
    OP = mybir.AluOpType
    AX = mybir.AxisListType
    ET = mybir.EngineType
    ds = bass.ds

    nc = bacc.Bacc("TRN2", target_bir_lowering=False, debug=False, num_devices=N_CORES)
    srcT = nc.dram_tensor("srcT", [BEX, H, S], f32, kind="ExternalInput").ap()
    srcTb = nc.dram_tensor("srcTb", [BEX, H, S], bf16, kind="ExternalInput").ap()
    tgtT = nc.dram_tensor("tgtT", [NH, 128, BEX], f32, kind="ExternalInput").ap()
    wp = nc.dram_tensor("wp", [H, H], f32, kind="ExternalInput").ap()
    vp = nc.dram_tensor("vp", [1, H], f32, kind="ExternalInput").ap()
    bp = nc.dram_tensor("bp", [1, H], f32, kind="ExternalInput").ap()
    bv = nc.dram_tensor("bv", [1, 1], f32, kind="ExternalInput").ap()
    out = nc.dram_tensor("out", [BEX, 128, NH], f32, kind="ExternalOutput").ap()
    scr_sp = nc.dram_tensor("scr_sp", [BEX, 1], f32).ap()

    with tile.TileContext(nc) as tc:
        with (
            tc.tile_pool(name="cpool", bufs=1) as cpool,
            tc.tile_pool(name="spool", bufs=5) as spool,
            tc.tile_pool(name="winpool", bufs=10) as winpool,
            tc.tile_pool(name="mpool", bufs=2) as mpool,
            tc.tile_pool(name="psB", bufs=1, space="PSUM") as psB,
        ):
            # ---------------- setup: batched small DMAs ----------------------
            tTall = cpool.tile([128, NH, BEX], f32, tag="tTall")
            nc.sync.dma_start(tTall[:], tgtT.rearrange("c p e -> p c e"))
            tT = [tTall[:, c, :] for c in range(NH)]

            wtall = cpool.tile([128, NH, H], f32, tag="wtall")
            nc.scalar.dma_start(
                wtall[:], wp.rearrange("(c p) j -> p c j", p=128)
            )
            wts = [wtall[:, c, :] for c in range(NH)]

            bp_sb = cpool.tile([BEX, H], f32, tag="bp_sb")
            v_b = cpool.tile([BEX, H], f32, tag="v_b")
            bv_sb = cpool.tile([BEX, 1], f32, tag="bv_sb")
            for e in range(BEX):
                nc.sync.dma_start(bp_sb[e : e + 1, :], bp[0:1, :])
                nc.sync.dma_start(v_b[e : e + 1, :], vp[0:1, :])
                nc.sync.dma_start(bv_sb[e : e + 1, :], bv[0:1, :])

            zeros = cpool.tile([128, 128], f32, tag="zeros")
            nc.vector.memset(zeros[:], 0.0)
            ebias = cpool.tile([128, 1], f32, tag="ebias")
            nc.vector.memset(ebias[:], EBIAS)
            t_rep32 = []
            t_rep16 = []
            for e in range(BEX):
                r32s, r16s = [], []
                for c in range(NH):
                    r32 = cpool.tile([128, 128], f32, tag=f"t_rep32_{e}_{c}")
                    nc.scalar.activation(
                        r32[:], zeros[:], AF.Identity, bias=tT[c][:, e : e + 1], scale=1.0
                    )
                    r16 = cpool.tile([128, 128], bf16, tag=f"t_rep16_{e}_{c}")
                    nc.vector.tensor_copy(r16[:], r32[:])
                    r32s.append(r32)
                    r16s.append(r16)
                t_rep32.append(r32s)
                t_rep16.append(r16s)

            iota_i = cpool.tile([128, WIN], i32, tag="iota_i")
            nc.gpsimd.iota(iota_i[:], pattern=[[1, WIN]], base=0, channel_multiplier=0)
            iota_f = cpool.tile([128, WIN], f32, tag="iota_f")
            nc.vector.tensor_copy(iota_f[:], iota_i[:])

            def emit_score_mms(e, ps, c, big):
                for k in range(NSB):
                    j, half = divmod(k, 2)
                    pslice = ps[j][64 * half : 64 * (half + 1), :]
                    nc.tensor.matmul(
                        pslice,
                        t_rep16[e][c][:, 0:64],
                        big[:, k * 512 : (k + 1) * 512],
                        start=(c == 0),
                        stop=(c == NH - 1),
                        tile_position=(0, 64 * half),
                        skip_group_check=True,
                    )

            # ---------------- ex0 scores interleaved with phase-0 matmuls ----
            ps_hp0 = psB.tile([BEX, 512], f32, tag="hp0", name="hp0")
            ps_hp1 = psB.tile([BEX, 512], f32, tag="hp1", name="hp1")
            ps0 = [
                psB.tile([128, 512], f32, tag=f"scA{j}", name=f"scA{j}_0")
                for j in range(NBK)
            ]
            for c in range(NH):
                big = spool.tile([128, S], bf16, tag="stream", name=f"big_0_{c}")
                nc.sync.dma_start(big[:], srcTb[0, c * 128 : (c + 1) * 128, :])
                emit_score_mms(0, ps0, c, big)
                nc.tensor.matmul(
                    ps_hp0[:], tT[c][:], wts[c][:, 0:512], start=(c == 0), stop=(c == NH - 1)
                )
                nc.tensor.matmul(
                    ps_hp1[:], tT[c][:], wts[c][:, 512:1024], start=(c == 0), stop=(c == NH - 1)
                )

            # ---------------- phase 0 tail: p, s0, gaussian ------------------
            hp_sb = cpool.tile([BEX, H], f32, tag="hp_sb")
            nc.vector.tensor_tensor(hp_sb[:, 0:512], ps_hp0[:], bp_sb[:, 0:512], OP.add)
            nc.vector.tensor_tensor(hp_sb[:, 512:1024], ps_hp1[:], bp_sb[:, 512:1024], OP.add)
            nc.scalar.activation(hp_sb[:], hp_sb[:], AF.Tanh)
            ttr_scr = cpool.tile([BEX, H], f32, tag="ttr_scr")
            pre = cpool.tile([BEX, 1], f32, tag="pre")
            nc.vector.tensor_tensor(ttr_scr[:], hp_sb[:], v_b[:], OP.mult)
            nc.vector.tensor_reduce(pre[:], ttr_scr[:], AX.X, OP.add)
            pv = cpool.tile([BEX, 1], f32, tag="pv")
            nc.scalar.activation(pv[:], pre[:], AF.Sigmoid, bias=bv_sb[:], scale=1.0)
            nc.vector.tensor_scalar(pv[:], pv[:], float(S), None, OP.mult)

            s0f = cpool.tile([BEX, 1], f32, tag="s0f")
            nc.vector.tensor_scalar(s0f[:], pv[:], float(WIN // 2), None, OP.subtract)
            nc.vector.tensor_scalar(s0f[:], s0f[:], 0.0, S0MAX, OP.max, OP.min)
            s0i = cpool.tile([BEX, 1], i32, tag="s0i")
            nc.vector.tensor_copy(s0i[:], s0f[:])
            s0ff = cpool.tile([BEX, 1], f32, tag="s0ff")
            nc.vector.tensor_copy(s0ff[:], s0i[:])

            spd = cpool.tile([BEX, 1], f32, tag="spd")
            nc.vector.tensor_tensor(spd[:], s0ff[:], pv[:], OP.subtract)
            nc.sync.dma_start(scr_sp[:], spd[:])

            s0_regs = []
            for e in range(BEX):
                s0_regs.append(
                    nc.values_load(
                        s0i[e : e + 1, 0:1],
                        engines=[ET.SP, ET.Activation],
                        min_val=0,
                        max_val=int(S0MAX),
                        skip_runtime_bounds_check=True,
                    )
                )

            gauss = []
            for e in range(BEX):
                sp_b = cpool.tile([128, 1], f32, tag=f"sp_b{e}")
                nc.sync.dma_start(sp_b[:], scr_sp[e : e + 1, 0:1].to_broadcast((128, 1)))
                d = mpool.tile([128, WIN], f32, tag="d", name=f"d_{e}")
                nc.vector.tensor_scalar(d[:], iota_f[:], sp_b[:], None, OP.add)
                nc.scalar.activation(d[:], d[:], AF.Square)
                g = cpool.tile([128, WIN], f32, tag=f"gauss{e}")
                nc.scalar.activation(g[:], d[:], AF.Exp, scale=GEXP)
                gauss.append(g)

            # ---------------- shared phase helpers ---------------------------
            def scores_phase(e):
                ps = [
                    psB.tile([128, 512], f32, tag=f"scA{j}", name=f"scA{j}_{e}")
                    for j in range(NBK)
                ]
                for c in range(NH):
                    big = spool.tile([128, S], bf16, tag="stream", name=f"big_{e}_{c}")
                    nc.sync.dma_start(big[:], srcTb[e, c * 128 : (c + 1) * 128, :])
                    emit_score_mms(e, ps, c, big)
                return ps

            def stats_phase(e, ps):
                # softmax denominator over full S (constant shift, no max);
                # column j of sums4 holds block 2j sums in rows 0:64 and
                # block 2j+1 sums in rows 64:128.
                sums4 = mpool.tile([128, NBK], f32, tag="sums4", name=f"sums4_{e}")
                for j in range(NBK):
                    ej = mpool.tile([128, 512], f32, tag="expjunk", name=f"ej_{e}_{j}")
                    nc.scalar.activation(
                        ej[:],
                        ps[j][:],
                        AF.Exp,
                        bias=ebias[:],
                        scale=SCALE,
                        accum_out=sums4[:, j : j + 1],
                    )
                z4 = mpool.tile([128, 1], f32, tag="z4", name=f"z4_{e}")
                nc.vector.tensor_reduce(z4[:], sums4[:], AX.X, OP.add)
                zsw = mpool.tile([128, 1], f32, tag="zsw", name=f"zsw_{e}")
                nc.sync.dma_start(zsw[0:64, :], z4[64:128, :])
                nc.sync.dma_start(zsw[64:128, :], z4[0:64, :])
                zf = mpool.tile([128, 1], f32, tag="zf", name=f"zf_{e}")
                nc.vector.tensor_tensor(zf[:], z4[:], zsw[:], OP.add)
                rz = mpool.tile([128, 1], f32, tag="rz", name=f"rz_{e}", bufs=2)
                nc.vector.reciprocal(rz[:], zf[:])
                return rz

            def window_phase(e, s0_reg, gauss_e, rz):
                # fp32 window: re-fetch window columns, recompute their scores
                # in fp32 (constant shift cancels against Z), then context.
                wins = []
                psw = psB.tile([128, WIN], f32, tag="psw", name=f"win_ps_{e}", bufs=2)
                for cc in range(NH // 2):
                    winp = winpool.tile(
                        [128, 2, WIN], f32, tag="win", name=f"win_{e}_{cc}", bufs=5
                    )
                    nc.scalar.dma_start(
                        winp[:],
                        srcT[e, 256 * cc : 256 * (cc + 1), ds(s0_reg, WIN)].rearrange(
                            "(c p) w -> p c w", p=128
                        ),
                    )
                    wins.extend([winp[:, 0, :], winp[:, 1, :]])
                for c in range(NH):
                    nc.tensor.matmul(
                        psw[:],
                        t_rep32[e][c][:],
                        wins[c],
                        start=(c == 0),
                        stop=(c == NH - 1),
                    )

                expw = mpool.tile([128, WIN], f32, tag="expw", name=f"expw_{e}")
                nc.scalar.activation(expw[:], psw[:], AF.Exp, bias=ebias[:], scale=SCALE)
                attnw = mpool.tile([128, WIN], f32, tag="attnw", name=f"attnw_{e}")
                nc.vector.tensor_tensor(attnw[:], expw[:], gauss_e[:], OP.mult)

                ctx = mpool.tile([128, NH], f32, tag="ctx", name=f"ctx_{e}")
                for c in range(NH):
                    scr = mpool.tile(
                        [128, WIN], f32, tag="scr512", name=f"scr_{e}_{c}", bufs=4
                    )
                    if c % 2 == 0:
                        nc.vector.tensor_tensor(scr[:], wins[c], attnw[:], OP.mult)
                        ejc = mpool.tile(
                            [128, WIN], f32, tag="ctxjunk", name=f"cj_{e}_{c}", bufs=2
                        )
                        nc.scalar.activation(
                            ejc[:], scr[:], AF.Identity, accum_out=ctx[:, c : c + 1]
                        )
                    else:
                        nc.gpsimd.tensor_tensor(scr[:], wins[c], attnw[:], OP.mult)
                        nc.vector.tensor_reduce(ctx[:, c : c + 1], scr[:], AX.X, OP.add)
                nc.vector.tensor_scalar(ctx[:], ctx[:], rz[:], None, OP.mult)
                nc.scalar.dma_start(out[e], ctx[:])

            rz0 = stats_phase(0, ps0)
            window_phase(0, s0_regs[0], gauss[0], rz0)
            ps1 = scores_phase(1)
            rz1 = stats_phase(1, ps1)
            window_phase(1, s0_regs[1], gauss[1], rz1)

    nc.compile()
    return nc


def _get_nc():
    if "nc" not in _CACHE:
        _CACHE["nc"] = _build()
    return _CACHE["nc"]


def _make_in_maps(src, tgt, wp, bp, vp, bv):
    import ml_dtypes

    srcT = np.ascontiguousarray(src.transpose(0, 2, 1))  # [B, H, S]
    srcTb = srcT.astype(ml_dtypes.bfloat16)
    in_maps = []
    for k in range(N_CORES):
        lo, hi = k * BEX, (k + 1) * BEX
        tgtT = np.ascontiguousarray(
            tgt[lo:hi].reshape(BEX, NH, 128).transpose(1, 2, 0)
        )  # [NH, 128, BEX]
        in_maps.append(
            {
                "srcT": srcT[lo:hi],
                "srcTb": srcTb[lo:hi],
                "tgtT": tgtT,
                "wp": wp,
                "vp": vp,
                "bp": bp,
                "bv": bv,
            }
        )
    return in_maps


def kernel(source_hidden_states, target_hidden_state, W_p, b_p, v_p, b_v):
    from concourse.bass_utils import run_bass_kernel_spmd

    src = np.asarray(source_hidden_states, dtype=np.float32)
    tgt = np.asarray(target_hidden_state, dtype=np.float32)
    wp = np.asarray(W_p, dtype=np.float32)
    bp = np.asarray(b_p, dtype=np.float32).reshape(1, H)
    vp = np.asarray(v_p, dtype=np.float32).reshape(1, H)
    bv = np.asarray(b_v, dtype=np.float32).reshape(1, 1)

    nc = _get_nc()
    in_maps = _make_in_maps(src, tgt, wp, bp, vp, bv)
    r = run_bass_kernel_spmd(nc, in_maps, list(range(N_CORES)))
    # out[e] is ctx [128, NH]; context[b, h] with h = c*128 + p lives at
    # out[b, p, c] -> transpose to [NH, 128] then flatten.
    outs = [
        r.results[k]["out"].transpose(0, 2, 1).reshape(BEX, H) for k in range(N_CORES)
    ]
    return np.concatenate(outs, axis=0)


# revision 24
# speedup vs baseline: 2.1684x; 1.0093x over previous
"""Luong local-p attention (scaled-dot, gaussian window) on 8 trn2 cores.

Strategy (data-parallel over batch, 2 examples/core):
  - Host: transpose source_hidden_states to [H, S] per example so the score
    matmul can contract over H on the TensorEngine with the target vector
    replicated as the stationary operand (scores come out replicated across
    the output partitions, which is exactly the layout the windowed context
    multiply needs). Ships a bf16 copy (streamed once for scores/softmax
    denominator) and keeps the fp32 copy for the window re-read.
  - Device per example:
      p = S*sigmoid(v_p . tanh(W_p^T t + b_p) + b_v)   (fp32 PE matmul + ACT)
      scores[s] = (src[s,:] . t)/sqrt(H)                (bf16 PE, psum-acc)
      softmax denominator Z over full S with a CONSTANT shift of -8 instead
      of the max (scores are ~N(0,1); fp32 range makes a computed max
      unnecessary, and the constant shift cancels exactly in the ratio)
      window [s0, s0+320), s0 = clamp(floor(p)-160, 0, S-320) covers every
      position whose gaussian factor exceeds ~3.7e-6 (5 sigma); window
      scores are recomputed in fp32 from the re-fetched fp32 window columns
      so the attention weights that matter are fp32-accurate. Context =
      windowed multiply-reduce spread across GPSIMD/DVE (multiplies) and
      ACT/DVE (free-dim reductions).
  - Resources: score psum uses 4 banks (two s-blocks per bank, 64-row
    replication via PE column tiling), the p-computation owns 2 banks and
    its 16 fp32 matmuls interleave into the DMA-gaps of example 0's score
    groups, the window recompute owns its own double-buffered bank. Window
    and output DMAs dispatch from the Activation queue to unload SP.
"""

import numpy as np

N_CORES = 8
B, S, H = 16, 4096, 1024
BEX = B // N_CORES  # examples per core
NH = H // 128  # h-chunks of 128 partitions
NSB = S // 512  # s-blocks of 512
NBK = NSB // 2  # psum banks for scores (2 blocks per bank)
WIN = 320
SCALE = 1.0 / 32.0  # 1/sqrt(H)
GEXP = -1.0 / 2048.0  # -1/(2*sigma^2), sigma = WINDOW/2 = 32
EBIAS = -8.0  # constant softmax shift
S0MAX = float(S - WIN)

_CACHE = {}


def _build():
    import concourse.bacc as bacc
    import concourse.bass as bass
    import concourse.mybir as mybir
    import concourse.tile as tile

    f32 = mybir.dt.float32
    bf16 = mybir.dt.bfloat16
    i32 = mybir.dt.int32
    AF = mybir.ActivationFunctionType
    OP = mybir.AluOpType
    AX = mybir.AxisListType
    ET = mybir.EngineType
    ds = bass.ds

    nc = bacc.Bacc("TRN2", target_bir_lowering=False, debug=False, num_devices=N_CORES)
    srcT = nc.dram_tensor("srcT", [BEX, H, S], f32, kind="ExternalInput").ap()
    srcTb = nc.dram_tensor("srcTb", [BEX, H, S], bf16, kind="ExternalInput").ap()
    tgtT = nc.dram_tensor("tgtT", [NH, 128, BEX], f32, kind="ExternalInput").ap()
    wp = nc.dram_tensor("wp", [H, H], f32, kind="ExternalInput").ap()
    vp = nc.dram_tensor("vp", [1, H], f32, kind="ExternalInput").ap()
    bp = nc.dram_tensor("bp", [1, H], f32, kind="ExternalInput").ap()
    bv = nc.dram_tensor("bv", [1, 1], f32, kind="ExternalInput").ap()
    out = nc.dram_tensor("out", [BEX, 128, NH], f32, kind="ExternalOutput").ap()
    scr_sp = nc.dram_tensor("scr_sp", [BEX, 1], f32).ap()

    with tile.TileContext(nc) as tc:
        with (
            tc.tile_pool(name="cpool", bufs=1) as cpool,
            tc.tile_pool(name="spool", bufs=5) as spool,
            tc.tile_pool(name="winpool", bufs=10) as winpool,
            tc.tile_pool(name="mpool", bufs=2) as mpool,
            tc.tile_pool(name="psB", bufs=1, space="PSUM") as psB,
        ):
            # ---------------- setup: batched small DMAs ----------------------
            tTall = cpool.tile([128, NH, BEX], f32, tag="tTall")
            nc.sync.dma_start(tTall[:], tgtT.rearrange("c p e -> p c e"))
            tT = [tTall[:, c, :] for c in range(NH)]

            wtall = cpool.tile([128, NH, H], f32, tag="wtall")
            for c in range(NH):
                nc.scalar.dma_start(wtall[:, c, :], wp[c * 128 : (c + 1) * 128, :])
            wts = [wtall[:, c, :] for c in range(NH)]

            bp_sb = cpool.tile([BEX, H], f32, tag="bp_sb")
            v_b = cpool.tile([BEX, H], f32, tag="v_b")
            bv_sb = cpool.tile([BEX, 1], f32, tag="bv_sb")
            for e in range(BEX):
                nc.sync.dma_start(bp_sb[e : e + 1, :], bp[0:1, :])
                nc.sync.dma_start(v_b[e : e + 1, :], vp[0:1, :])
                nc.sync.dma_start(bv_sb[e : e + 1, :], bv[0:1, :])

            zeros = cpool.tile([128, 128], f32, tag="zeros")
            nc.vector.memset(zeros[:], 0.0)
            ebias = cpool.tile([128, 1], f32, tag="ebias")
            nc.vector.memset(ebias[:], EBIAS)
            t_rep32 = []
            t_rep16 = []
            for e in range(BEX):
                r32s, r16s = [], []
                for c in range(NH):
                    r32 = cpool.tile([128, 128], f32, tag=f"t_rep32_{e}_{c}")
                    nc.scalar.activation(
                        r32[:], zeros[:], AF.Identity, bias=tT[c][:, e : e + 1], scale=1.0
                    )
                    r16 = cpool.tile([128, 128], bf16, tag=f"t_rep16_{e}_{c}")
                    nc.vector.tensor_copy(r16[:], r32[:])
                    r32s.append(r32)
                    r16s.append(r16)
                t_rep32.append(r32s)
                t_rep16.append(r16s)

            iota_i = cpool.tile([128, WIN], i32, tag="iota_i")
            nc.gpsimd.iota(iota_i[:], pattern=[[1, WIN]], base=0, channel_multiplier=0)
            iota_f = cpool.tile([128, WIN], f32, tag="iota_f")
            nc.vector.tensor_copy(iota_f[:], iota_i[:])

            def emit_score_mms(e, ps, c, big):
                for k in range(NSB):
                    j, half = divmod(k, 2)
                    pslice = ps[j][64 * half : 64 * (half + 1), :]
                    nc.tensor.matmul(
                        pslice,
                        t_rep16[e][c][:, 0:64],
                        big[:, k * 512 : (k + 1) * 512],
                        start=(c == 0),
                        stop=(c == NH - 1),
                        tile_position=(0, 64 * half),
                        skip_group_check=True,
                    )

            # ---------------- ex0 scores interleaved with phase-0 matmuls ----
            ps_hp0 = psB.tile([BEX, 512], f32, tag="hp0", name="hp0")
            ps_hp1 = psB.tile([BEX, 512], f32, tag="hp1", name="hp1")
            ps0 = [
                psB.tile([128, 512], f32, tag=f"scA{j}", name=f"scA{j}_0")
                for j in range(NBK)
            ]
            for c in range(NH):
                big = spool.tile([128, S], bf16, tag="stream", name=f"big_0_{c}")
                nc.sync.dma_start(big[:], srcTb[0, c * 128 : (c + 1) * 128, :])
                emit_score_mms(0, ps0, c, big)
                nc.tensor.matmul(
                    ps_hp0[:], tT[c][:], wts[c][:, 0:512], start=(c == 0), stop=(c == NH - 1)
                )
                nc.tensor.matmul(
                    ps_hp1[:], tT[c][:], wts[c][:, 512:1024], start=(c == 0), stop=(c == NH - 1)
                )

            # ---------------- phase 0 tail: p, s0, gaussian ------------------
            hp_sb = cpool.tile([BEX, H], f32, tag="hp_sb")
            nc.vector.tensor_tensor(hp_sb[:, 0:512], ps_hp0[:], bp_sb[:, 0:512], OP.add)
            nc.vector.tensor_tensor(hp_sb[:, 512:1024], ps_hp1[:], bp_sb[:, 512:1024], OP.add)
            nc.scalar.activation(hp_sb[:], hp_sb[:], AF.Tanh)
            ttr_scr = cpool.tile([BEX, H], f32, tag="ttr_scr")
            pre = cpool.tile([BEX, 1], f32, tag="pre")
            nc.vector.tensor_tensor(ttr_scr[:], hp_sb[:], v_b[:], OP.mult)
            nc.vector.tensor_reduce(pre[:], ttr_scr[:], AX.X, OP.add)
            pv = cpool.tile([BEX, 1], f32, tag="pv")
            nc.scalar.activation(pv[:], pre[:], AF.Sigmoid, bias=bv_sb[:], scale=1.0)
            nc.vector.tensor_scalar(pv[:], pv[:], float(S), None, OP.mult)

            s0f = cpool.tile([BEX, 1], f32, tag="s0f")
            nc.vector.tensor_scalar(s0f[:], pv[:], float(WIN // 2), None, OP.subtract)
            nc.vector.tensor_scalar(s0f[:], s0f[:], 0.0, S0MAX, OP.max, OP.min)
            s0i = cpool.tile([BEX, 1], i32, tag="s0i")
            nc.vector.tensor_copy(s0i[:], s0f[:])
            s0ff = cpool.tile([BEX, 1], f32, tag="s0ff")
            nc.vector.tensor_copy(s0ff[:], s0i[:])

            spd = cpool.tile([BEX, 1], f32, tag="spd")
            nc.vector.tensor_tensor(spd[:], s0ff[:], pv[:], OP.subtract)
            nc.sync.dma_start(scr_sp[:], spd[:])

            s0_regs = []
            for e in range(BEX):
                s0_regs.append(
                    nc.values_load(
                        s0i[e : e + 1, 0:1],
                        engines=[ET.SP, ET.Activation],
                        min_val=0,
                        max_val=int(S0MAX),
                        skip_runtime_bounds_check=True,
                    )
                )

            gauss = []
            for e in range(BEX):
                sp_b = cpool.tile([128, 1], f32, tag=f"sp_b{e}")
                nc.sync.dma_start(sp_b[:], scr_sp[e : e + 1, 0:1].to_broadcast((128, 1)))
                d = mpool.tile([128, WIN], f32, tag="d", name=f"d_{e}")
                nc.vector.tensor_scalar(d[:], iota_f[:], sp_b[:], None, OP.add)
                nc.scalar.activation(d[:], d[:], AF.Square)
                g = cpool.tile([128, WIN], f32, tag=f"gauss{e}")
                nc.scalar.activation(g[:], d[:], AF.Exp, scale=GEXP)
                gauss.append(g)

            # ---------------- shared phase helpers ---------------------------
            def scores_phase(e):
                ps = [
                    psB.tile([128, 512], f32, tag=f"scA{j}", name=f"scA{j}_{e}")
                    for j in range(NBK)
                ]
                for c in range(NH):
                    big = spool.tile([128, S], bf16, tag="stream", name=f"big_{e}_{c}")
                    nc.sync.dma_start(big[:], srcTb[e, c * 128 : (c + 1) * 128, :])
                    emit_score_mms(e, ps, c, big)
                return ps

            def stats_phase(e, ps):
                # softmax denominator over full S (constant shift, no max);
                # column j of sums4 holds block 2j sums in rows 0:64 and
                # block 2j+1 sums in rows 64:128.
                sums4 = mpool.tile([128, NBK], f32, tag="sums4", name=f"sums4_{e}")
                for j in range(NBK):
                    ej = mpool.tile([128, 512], f32, tag="expjunk", name=f"ej_{e}_{j}")
                    nc.scalar.activation(
                        ej[:],
                        ps[j][:],
                        AF.Exp,
                        bias=ebias[:],
                        scale=SCALE,
                        accum_out=sums4[:, j : j + 1],
                    )
                z4 = mpool.tile([128, 1], f32, tag="z4", name=f"z4_{e}")
                nc.vector.tensor_reduce(z4[:], sums4[:], AX.X, OP.add)
                zsw = mpool.tile([128, 1], f32, tag="zsw", name=f"zsw_{e}")
                nc.sync.dma_start(zsw[0:64, :], z4[64:128, :])
                nc.sync.dma_start(zsw[64:128, :], z4[0:64, :])
                zf = mpool.tile([128, 1], f32, tag="zf", name=f"zf_{e}")
                nc.vector.tensor_tensor(zf[:], z4[:], zsw[:], OP.add)
                rz = mpool.tile([128, 1], f32, tag="rz", name=f"rz_{e}", bufs=2)
                nc.vector.reciprocal(rz[:], zf[:])
                return rz

            def win_dma_phase(e, s0_reg):
                wins = []
                for cc in range(NH // 2):
                    winp = winpool.tile(
                        [128, 2, WIN], f32, tag="win", name=f"win_{e}_{cc}", bufs=10
                    )
                    nc.scalar.dma_start(
                        winp[:],
                        srcT[e, 256 * cc : 256 * (cc + 1), ds(s0_reg, WIN)].rearrange(
                            "(c p) w -> p c w", p=128
                        ),
                    )
                    wins.extend([winp[:, 0, :], winp[:, 1, :]])
                return wins

            def window_phase(e, wins, gauss_e, rz):
                # fp32 window: recompute window scores in fp32 (constant shift
                # cancels against Z), then context.
                psw = psB.tile([128, WIN], f32, tag="psw", name=f"win_ps_{e}", bufs=2)
                for c in range(NH):
                    nc.tensor.matmul(
                        psw[:],
                        t_rep32[e][c][:],
                        wins[c],
                        start=(c == 0),
                        stop=(c == NH - 1),
                    )

                expw = mpool.tile([128, WIN], f32, tag="expw", name=f"expw_{e}")
                nc.scalar.activation(expw[:], psw[:], AF.Exp, bias=ebias[:], scale=SCALE)
                attnw = mpool.tile([128, WIN], f32, tag="attnw", name=f"attnw_{e}")
                nc.vector.tensor_tensor(attnw[:], expw[:], gauss_e[:], OP.mult)

                ctx = mpool.tile([128, NH], f32, tag="ctx", name=f"ctx_{e}")
                for c in range(NH):
                    scr = mpool.tile(
                        [128, WIN], f32, tag="scr512", name=f"scr_{e}_{c}", bufs=4
                    )
                    if c % 2 == 0:
                        nc.vector.tensor_tensor(scr[:], wins[c], attnw[:], OP.mult)
                        ejc = mpool.tile(
                            [128, WIN], f32, tag="ctxjunk", name=f"cj_{e}_{c}", bufs=2
                        )
                        nc.scalar.activation(
                            ejc[:], scr[:], AF.Identity, accum_out=ctx[:, c : c + 1]
                        )
                    else:
                        nc.gpsimd.tensor_tensor(scr[:], wins[c], attnw[:], OP.mult)
                        nc.vector.tensor_reduce(ctx[:, c : c + 1], scr[:], AX.X, OP.add)
                nc.vector.tensor_scalar(ctx[:], ctx[:], rz[:], None, OP.mult)
                nc.scalar.dma_start(out[e], ctx[:])

            wins0 = win_dma_phase(0, s0_regs[0])
            wins1 = win_dma_phase(1, s0_regs[1])
            rz0 = stats_phase(0, ps0)
            window_phase(0, wins0, gauss[0], rz0)
            ps1 = scores_phase(1)
            rz1 = stats_phase(1, ps1)
            window_phase(1, wins1, gauss[1], rz1)

    nc.compile()
    return nc


def _get_nc():
    if "nc" not in _CACHE:
        _CACHE["nc"] = _build()
    return _CACHE["nc"]


def _make_in_maps(src, tgt, wp, bp, vp, bv):
    import ml_dtypes

    srcT = np.ascontiguousarray(src.transpose(0, 2, 1))  # [B, H, S]
    srcTb = srcT.astype(ml_dtypes.bfloat16)
    in_maps = []
    for k in range(N_CORES):
        lo, hi = k * BEX, (k + 1) * BEX
        tgtT = np.ascontiguousarray(
            tgt[lo:hi].reshape(BEX, NH, 128).transpose(1, 2, 0)
        )  # [NH, 128, BEX]
        in_maps.append(
            {
                "srcT": srcT[lo:hi],
                "srcTb": srcTb[lo:hi],
                "tgtT": tgtT,
                "wp": wp,
                "vp": vp,
                "bp": bp,
                "bv": bv,
            }
        )
    return in_maps


def kernel(source_hidden_states, target_hidden_state, W_p, b_p, v_p, b_v):
    from concourse.bass_utils import run_bass_kernel_spmd

    src = np.asarray(source_hidden_states, dtype=np.float32)
    tgt = np.asarray(target_hidden_state, dtype=np.float32)
    wp = np.asarray(W_p, dtype=np.float32)
    bp = np.asarray(b_p, dtype=np.float32).reshape(1, H)
    vp = np.asarray(v_p, dtype=np.float32).reshape(1, H)
    bv = np.asarray(b_v, dtype=np.float32).reshape(1, 1)

    nc = _get_nc()
    in_maps = _make_in_maps(src, tgt, wp, bp, vp, bv)
    r = run_bass_kernel_spmd(nc, in_maps, list(range(N_CORES)))
    # out[e] is ctx [128, NH]; context[b, h] with h = c*128 + p lives at
    # out[b, p, c] -> transpose to [NH, 128] then flatten.
    outs = [
        r.results[k]["out"].transpose(0, 2, 1).reshape(BEX, H) for k in range(N_CORES)
    ]
    return np.concatenate(outs, axis=0)
